# revision 19
# baseline (speedup 1.0000x reference)
"""Trainium2 Bass kernel for nn_DiffusionActionHead (B=8, S=2048, D=4096).

Strategy (8 NeuronCores):
  - Data-parallel over batch for everything touching llm_output (32 MiB/core).
  - Tensor-parallel weight reads: core i reads column-slice i of wq/wk/wv,
    row-slice i of wo, column/row slice i of mlp_w1/mlp_w2 (~96 MiB of
    weights split 8 ways), tiny diffusion tail replicated.
  - MAP-head attention with q_len=1 is collapsed algebraically:
        scores[s,h] = llm[s,:] . U[:,h],   U[:,h] = wk[:,hb] @ q_h / sqrt(DH)
        pooled[h,:] = softmax(scores)[h,:] @ llm
        ctx[hb]     = pooled[h,:] @ wv[:,hb] + bv[hb]
    (bk shifts scores by a per-head constant -> cancels in softmax.)
  - 4 small collectives: AllGather(U cols), AllToAll(pooled, head<->batch),
    AllReduce(attn_out partial), AllReduce(mlp partial).
Activations are kept feature-on-partition ("transposed") so every big matmul
streams the weight slice in its natural DRAM layout as the moving operand.
"""

import numpy as np
import ml_dtypes
import sys

if "/opt/trn_rl_repo" not in sys.path:
    sys.path.insert(0, "/opt/trn_rl_repo")

import concourse.bass as bass
import concourse.tile as tile
from concourse import bacc, mybir
from concourse.masks import make_identity
from concourse.bass_utils import run_bass_kernel_spmd

F32 = mybir.dt.float32
F32R = mybir.dt.float32r
BF16 = mybir.dt.bfloat16
AF = mybir.ActivationFunctionType
ALU = mybir.AluOpType

B, S, D = 8, 2048, 4096
H, AD, TD, HID, NBLK = 8, 7, 32, 256, 3
DH = D // H            # 512
NC = 8                 # cores
P = 128
SC = S // P            # 16 S-chunks
DC = D // P            # 32 D-chunks
HD2 = D // 2           # 2048 (half width for 4-bank PSUM tiles)
F1S = 4 * D // NC      # 2048 per-core hidden cols of mlp_w1
RSQRT_DH = 1.0 / float(np.sqrt(DH))
TWO_PI = 2.0 * float(np.pi)

# matmul dtype knob per family: "f32" (exact, 4 cyc/row) or "f32r" (fast).
MM_KNOB = {
    "q": "f32", "u": "f32", "pooled": "f32", "ctx": "f32",
    "attn": "f32", "mm1": "f32", "mm2": "f32", "rin": "f32",
    "tail": "f32",
}


def _mm(ap, fam):
    if MM_KNOB[fam] == "f32r" and ap.dtype == F32:
        return ap.bitcast(F32R)
    return ap


def _bcast(src_ap, nparts):
    """Partition-broadcast a (1, N) DRAM AP to (nparts, N)."""
    ap = src_ap
    assert ap.shape[0] == 1, ap.shape
    return bass.AP(tensor=ap.tensor, offset=ap.offset,
                   ap=[[0, nparts]] + [list(x) for x in ap.ap[1:]])


def build_program():
    nc = bacc.Bacc("TRN2", target_bir_lowering=False, debug=False,
                   num_devices=NC)

    t = {}

    def din(name, shape, dtype=F32):
        t[name] = nc.dram_tensor(name, shape, dtype, kind="ExternalInput")

    din("llm", [S, D]); din("llmT", [D, S], BF16)
    din("wq_s", [D, DH]); din("bq_s", [1, DH])
    din("wkT_s", [DH, D])
    din("wv_s", [D, DH]); din("bv_s", [1, DH])
    din("wo_s", [DH, D]); din("bo_r", [1, D])
    din("ln_g_r", [P, DC]); din("ln_b_r", [P, DC])
    din("w1_s", [D, F1S]); din("b1_s", [1, F1S])
    din("w2_s", [F1S, D]); din("b2_r", [1, D])
    din("probe_r", [P, DC])
    din("four_w2", [TD, 1]); din("phase2", [TD, 1])
    din("timeT", [1, B]); din("naT", [AD, B])
    din("cond_w1", [TD, 2 * TD]); din("cond_b1c", [2 * TD, 1])
    din("cond_w2", [2 * TD, TD]); din("cond_b2c", [TD, 1])
    din("rin_cond", [TD, HID]); din("rin_pool", [D, HID])
    din("rin_na", [AD, HID]); din("rin_b", [1, HID])
    din("blk_ln_g", [NBLK, HID]); din("blk_ln_b", [NBLK, HID])
    din("blk_w1", [NBLK, HID, 4 * HID]); din("blk_b1", [NBLK, 4 * HID])
    din("blk_w2", [NBLK, 4 * HID, HID]); din("blk_b2", [NBLK, HID])
    din("out_w", [HID, AD]); din("out_bc", [1, AD])
    t["res"] = nc.dram_tensor("res", [B, AD], F32, kind="ExternalOutput")

    # collective bounce buffers (internal DRAM; outputs in Shared space)
    t["cc_u_in"] = nc.dram_tensor("cc_u_in", [1, D], F32)
    t["cc_u_out"] = nc.dram_tensor("cc_u_out", [NC, D], F32, addr_space="Shared")
    t["cc_pool_in"] = nc.dram_tensor("cc_pool_in", [H, D], F32)
    t["cc_pool_out"] = nc.dram_tensor("cc_pool_out", [B, D], F32)
    t["cc_attn_in"] = nc.dram_tensor("cc_attn_in", [B, D], F32)
    t["cc_attn_out"] = nc.dram_tensor("cc_attn_out", [B, D], F32,
                                      addr_space="Shared")
    t["cc_mlp_in"] = nc.dram_tensor("cc_mlp_in", [B, D], F32)
    t["cc_mlp_out"] = nc.dram_tensor("cc_mlp_out", [B, D], F32,
                                     addr_space="Shared")

    with tile.TileContext(nc) as tc:
        import contextlib
        with contextlib.ExitStack() as ctx:
            _build(nc, tc, t, ctx)
    nc.finalize()
    return nc


def _build(nc, tc, t, ctx):
    GROUPS = [list(range(NC))]

    singles = ctx.enter_context(tc.tile_pool(name="singles", bufs=1))
    llm_pool = ctx.enter_context(tc.tile_pool(name="llm_pool", bufs=3))
    llmT_pool = ctx.enter_context(tc.tile_pool(name="llmT_pool", bufs=2))
    wst = ctx.enter_context(tc.tile_pool(name="wst", bufs=4))
    nat16 = ctx.enter_context(tc.tile_pool(name="nat16", bufs=2))
    nat8 = ctx.enter_context(tc.tile_pool(name="nat8", bufs=2))
    bcp = ctx.enter_context(tc.tile_pool(name="bcp", bufs=1))
    psA = ctx.enter_context(tc.tile_pool(name="psA", bufs=1, space="PSUM"))
    psB = ctx.enter_context(tc.tile_pool(name="psB", bufs=2, space="PSUM"))
    psC = ctx.enter_context(tc.tile_pool(name="psC", bufs=2, space="PSUM"))

    ident = singles.tile([P, P], F32)
    make_identity(nc, ident)
    eps_sb = singles.tile([P, 1], F32)
    nc.vector.memset(eps_sb[:], 1e-5)

    def evict(dst, src):
        nc.any.tensor_copy(out=dst, in_=src)

    def t_nat_to_T(src_nat, dst_T, nchunks, npart, uid):
        """(npart, nchunks*128) sbuf -> (128, nchunks, npart) sbuf via PE."""
        for c in range(nchunks):
            ps = psB.tile([P, 8], F32, tag="tp8", name=f"tp_{uid}_{c}")
            nc.tensor.transpose(ps[:, :npart], src_nat[:, c * P:(c + 1) * P],
                                ident[:npart, :npart])
            evict(dst_T[:, c, :], ps[:, :npart])

    def layernorm_nat(x_nat, npart, n, y_nat, uid, eps=1e-5):
        """y = (x - mean) / sqrt(var + eps) over free dim of (npart, n)."""
        nsub = max(1, n // 512)
        st = nat8.tile([npart, nsub, nc.vector.BN_STATS_DIM], F32, tag="lnst",
                       name=f"lnst_{uid}")
        xg = x_nat.rearrange("p (a b) -> p a b", a=nsub)
        for g in range(nsub):
            nc.vector.bn_stats(out=st[:, g, :], in_=xg[:, g, :])
        mv = nat8.tile([npart, nc.vector.BN_AGGR_DIM], F32, tag="lnmv",
                       name=f"lnmv_{uid}")
        nc.vector.bn_aggr(out=mv[:], in_=st[:])
        std = nat8.tile([npart, 1], F32, tag="lnsd", name=f"lnsd_{uid}")
        nc.scalar.activation(out=std[:], in_=mv[:, 1:2], func=AF.Sqrt,
                             bias=eps_sb[:npart, :])
        nc.vector.reciprocal(out=std[:], in_=std[:])
        nc.vector.tensor_scalar(out=y_nat, in0=x_nat, scalar1=mv[:, 0:1],
                                scalar2=std[:], op0=ALU.subtract, op1=ALU.mult)

    # =======================================================================
    # STEP 1: q_s = (probe @ wq_s + bq_s) / sqrt(DH)   -> (1, 512) natural
    # =======================================================================
    probe_sb = singles.tile([P, DC], F32)
    nc.sync.dma_start(out=probe_sb[:], in_=t["probe_r"][:])
    bq_sb = singles.tile([1, DH], F32)
    nc.sync.dma_start(out=bq_sb[:], in_=t["bq_s"][:])

    q_nat = singles.tile([1, DH], F32)
    ps_q = psC.tile([1, DH], F32, tag="vec", name="ps_q")
    for k in range(DC):
        wt = wst.tile([P, HD2], F32, tag="wst", name=f"wq_t{k}")
        nc.sync.dma_start(out=wt[:, :DH], in_=t["wq_s"][k * P:(k + 1) * P, :])
        nc.tensor.matmul(ps_q[:], _mm(probe_sb[:, k:k + 1], "q"),
                         _mm(wt[:, :DH], "q"),
                         start=(k == 0), stop=(k == DC - 1))
    nc.vector.tensor_add(out=q_nat[:], in0=ps_q[:], in1=bq_sb[:])
    nc.vector.tensor_scalar_mul(out=q_nat[:], in0=q_nat[:], scalar1=RSQRT_DH)

    qT = singles.tile([P, DH // P], F32)  # (128, 4)
    for c in range(DH // P):
        ps = psB.tile([P, 8], F32, tag="tp8", name=f"tp_q_{c}")
        nc.tensor.transpose(ps[:, :1], q_nat[:, c * P:(c + 1) * P], ident[:1, :1])
        evict(qT[:, c:c + 1], ps[:, :1])

    # =======================================================================
    # STEP 2: U column of this core's head: U = wkT_s.T @ q~  -> (1, 4096)
    #         AllGather -> cc_u_out (8, 4096) = U.T with one row per head
    # =======================================================================
    u_nat = singles.tile([1, D], F32)
    for nhalf in range(2):
        wk_tiles = []
        for k in range(DH // P):
            wt = wst.tile([P, HD2], F32, tag="wst", name=f"wk_t{nhalf}_{k}")
            nc.sync.dma_start(
                out=wt[:],
                in_=t["wkT_s"][k * P:(k + 1) * P, nhalf * HD2:(nhalf + 1) * HD2])
            wk_tiles.append(wt)
        for ncol in range(4):
            n0 = nhalf * 4 + ncol
            ps_u = psC.tile([1, DH], F32, tag="vec", name=f"ps_u_{n0}")
            for k in range(DH // P):
                nc.tensor.matmul(
                    ps_u[:], _mm(qT[:, k:k + 1], "u"),
                    _mm(wk_tiles[k][:, ncol * DH:(ncol + 1) * DH], "u"),
                    start=(k == 0), stop=(k == DH // P - 1))
            evict(u_nat[:, n0 * DH:(n0 + 1) * DH], ps_u[:])

    nc.gpsimd.dma_start(out=t["cc_u_in"][:], in_=u_nat[:])
    nc.gpsimd.collective_compute(
        "AllGather", ALU.bypass, replica_groups=GROUPS,
        ins=[t["cc_u_in"][:].opt()], outs=[t["cc_u_out"][:].opt()])

    # read back U.T (8, 4096), transpose to (128, 32, 8), cast to bf16
    uh_nat = nat16.tile([H, D], F32, tag="nat16", name="uh_nat")
    nc.sync.dma_start(out=uh_nat[:], in_=t["cc_u_out"][:])
    u_bf = singles.tile([P, DC, H], BF16)
    for c in range(DC):
        ps = psB.tile([P, 8], F32, tag="tp8", name=f"tp_u_{c}")
        nc.tensor.transpose(ps[:, :H], uh_nat[:, c * P:(c + 1) * P],
                            ident[:H, :H])
        evict(u_bf[:, c, :], ps[:, :H])

    # =======================================================================
    # STEP 3: scoresT (8, 2048) = U.T @ llmT  (bf16 inputs, fp32 accum)
    # =======================================================================
    ps_sc = psA.tile([H, S], F32, tag="big", name="ps_sc")
    for k in range(DC):
        lt = llmT_pool.tile([P, S], BF16, tag="llmT", name=f"llmT_t{k}")
        nc.sync.dma_start(out=lt[:], in_=t["llmT"][k * P:(k + 1) * P, :])
        for n in range(S // 512):
            nc.tensor.matmul(ps_sc[:, n * 512:(n + 1) * 512],
                             u_bf[:, k, :], lt[:, n * 512:(n + 1) * 512],
                             start=(k == 0), stop=(k == DC - 1))

    # =======================================================================
    # STEP 4: softmax over S. Max-subtraction is skipped deliberately:
    # softmax is shift-invariant and |scores| here is < ~1, so exp() is
    # perfectly conditioned; result is mathematically identical.
    # =======================================================================
    p_nat = nat8.tile([H, S], F32, tag="nat8", name="p_nat")
    nc.scalar.activation(out=p_nat[:], in_=ps_sc[:], func=AF.Exp)
    den = singles.tile([H, 1], F32)
    nc.vector.reduce_sum(out=den[:], in_=p_nat[:], axis=mybir.AxisListType.X)
    nc.vector.reciprocal(out=den[:], in_=den[:])
    nc.vector.tensor_scalar_mul(out=p_nat[:], in0=p_nat[:], scalar1=den[:])
    pT = singles.tile([P, SC, H], F32)
    t_nat_to_T(p_nat, pT, SC, H, "p")

    # =======================================================================
    # STEP 5: pooled (8, 4096) = pT.T @ llm ; AllToAll (head <-> batch)
    # =======================================================================
    pooled_nat = nat16.tile([H, D], F32, tag="nat16", name="pooled_nat")
    for half in range(2):
        ps_p = psA.tile([H, HD2], F32, tag="big", name=f"ps_pool_{half}")
        for s in range(SC):
            lt = llm_pool.tile([P, HD2], F32, tag="llm", name=f"llm_t{half}_{s}")
            nc.sync.dma_start(
                out=lt[:],
                in_=t["llm"][s * P:(s + 1) * P, half * HD2:(half + 1) * HD2])
            for n in range(HD2 // 512):
                nc.tensor.matmul(ps_p[:, n * 512:(n + 1) * 512],
                                 _mm(pT[:, s, :], "pooled"),
                                 _mm(lt[:, n * 512:(n + 1) * 512], "pooled"),
                                 start=(s == 0), stop=(s == SC - 1))
        evict(pooled_nat[:, half * HD2:(half + 1) * HD2], ps_p[:])

    nc.gpsimd.dma_start(out=t["cc_pool_in"][:], in_=pooled_nat[:])
    nc.gpsimd.collective_compute(
        "AllToAll", ALU.bypass, replica_groups=GROUPS,
        ins=[t["cc_pool_in"][:].opt()], outs=[t["cc_pool_out"][:].opt()])

    # =======================================================================
    # STEP 6: ctx for this core's head, all batches: (8, 512)
    # =======================================================================
    poolh_nat = nat16.tile([B, D], F32, tag="nat16", name="poolh_nat")
    nc.sync.dma_start(out=poolh_nat[:], in_=t["cc_pool_out"][:])
    poolhT = singles.tile([P, DC, B], F32)
    t_nat_to_T(poolh_nat, poolhT, DC, B, "ph")

    bv_bc = bcp.tile([B, D], F32, tag="bc", name="bv_bc")
    nc.gpsimd.dma_start(out=bv_bc[:, :DH], in_=_bcast(t["bv_s"][:], B))
    ps_cx = psA.tile([B, DH], F32, tag="big", name="ps_cx")
    for k in range(DC):
        wt = wst.tile([P, HD2], F32, tag="wst", name=f"wv_t{k}")
        nc.sync.dma_start(out=wt[:, :DH], in_=t["wv_s"][k * P:(k + 1) * P, :])
        nc.tensor.matmul(ps_cx[:], _mm(poolhT[:, k, :], "ctx"),
                         _mm(wt[:, :DH], "ctx"),
                         start=(k == 0), stop=(k == DC - 1))
    ctx_nat = nat8.tile([B, DH], F32, tag="nat8", name="ctx_nat")
    nc.vector.tensor_add(out=ctx_nat[:], in0=ps_cx[:], in1=bv_bc[:, :DH])
    ctxT = singles.tile([P, DH // P, B], F32)
    t_nat_to_T(ctx_nat, ctxT, DH // P, B, "cx")

    # =======================================================================
    # STEP 7: attn_out partial (8, 4096) = ctx @ wo_s ; AllReduce
    # =======================================================================
    attn_part = nat16.tile([B, D], F32, tag="nat16", name="attn_part")
    for half in range(2):
        ps_a = psA.tile([B, HD2], F32, tag="big", name=f"ps_attn_{half}")
        for k in range(DH // P):
            wt = wst.tile([P, HD2], F32, tag="wst", name=f"wo_t{half}_{k}")
            nc.sync.dma_start(
                out=wt[:],
                in_=t["wo_s"][k * P:(k + 1) * P, half * HD2:(half + 1) * HD2])
            for n in range(HD2 // 512):
                nc.tensor.matmul(ps_a[:, n * 512:(n + 1) * 512],
                                 _mm(ctxT[:, k, :], "attn"),
                                 _mm(wt[:, n * 512:(n + 1) * 512], "attn"),
                                 start=(k == 0), stop=(k == DH // P - 1))
        evict(attn_part[:, half * HD2:(half + 1) * HD2], ps_a[:])
    nc.gpsimd.dma_start(out=t["cc_attn_in"][:], in_=attn_part[:])
    nc.gpsimd.collective_compute(
        "AllReduce", ALU.add, replica_groups=GROUPS,
        ins=[t["cc_attn_in"][:].opt()], outs=[t["cc_attn_out"][:].opt()])

    # =======================================================================
    # STEP 8: attn_out = AR + bo ; y = LN(attn_out)*g+b ; mlp partial ; AR
    # =======================================================================
    attn_nat = singles.tile([B, D], F32)  # persists (residual)
    nc.sync.dma_start(out=attn_nat[:], in_=t["cc_attn_out"][:])
    bo_bc = bcp.tile([B, D], F32, tag="bc", name="bo_bc")
    nc.gpsimd.dma_start(out=bo_bc[:], in_=_bcast(t["bo_r"][:], B))
    nc.vector.tensor_add(out=attn_nat[:], in0=attn_nat[:], in1=bo_bc[:])

    y_nat = nat16.tile([B, D], F32, tag="nat16", name="y_nat")
    layernorm_nat(attn_nat[:], B, D, y_nat[:], "ln0")
    yT = singles.tile([P, DC, B], F32)
    t_nat_to_T(y_nat, yT, DC, B, "y")
    # LN affine in T layout (gamma/beta become per-partition scalars)
    lng_sb = singles.tile([P, DC], F32)
    nc.sync.dma_start(out=lng_sb[:], in_=t["ln_g_r"][:])
    lnb_sb = singles.tile([P, DC], F32)
    nc.sync.dma_start(out=lnb_sb[:], in_=t["ln_b_r"][:])
    for c in range(DC):
        nc.vector.tensor_scalar(out=yT[:, c, :], in0=yT[:, c, :],
                                scalar1=lng_sb[:, c:c + 1],
                                scalar2=lnb_sb[:, c:c + 1],
                                op0=ALU.mult, op1=ALU.add)

    # mm1: h1 (8, 2048) = y @ w1_s ; + b1 ; exact gelu
    ps_h1 = psA.tile([B, F1S], F32, tag="big", name="ps_h1")
    for k in range(DC):
        wt = wst.tile([P, F1S], F32, tag="wst", name=f"w1_t{k}")
        nc.sync.dma_start(out=wt[:], in_=t["w1_s"][k * P:(k + 1) * P, :])
        for n in range(F1S // 512):
            nc.tensor.matmul(ps_h1[:, n * 512:(n + 1) * 512],
                             _mm(yT[:, k, :], "mm1"),
                             _mm(wt[:, n * 512:(n + 1) * 512], "mm1"),
                             start=(k == 0), stop=(k == DC - 1))
    b1_bc = bcp.tile([B, F1S], F32, tag="bc2", name="b1_bc")
    nc.gpsimd.dma_start(out=b1_bc[:], in_=_bcast(t["b1_s"][:], B))
    g_nat = nat8.tile([B, F1S], F32, tag="nat8", name="g_nat")
    nc.vector.tensor_add(out=g_nat[:], in0=ps_h1[:], in1=b1_bc[:])
    nc.scalar.activation(out=g_nat[:], in_=g_nat[:], func=AF.Gelu)
    gT = singles.tile([P, F1S // P, B], F32)
    t_nat_to_T(g_nat, gT, F1S // P, B, "g")

    # mm2: h2 partial (8, 4096) = g @ w2_s ; AllReduce
    h2_nat = nat16.tile([B, D], F32, tag="nat16", name="h2_nat")
    for half in range(2):
        ps_h2 = psA.tile([B, HD2], F32, tag="big", name=f"ps_h2_{half}")
        for k in range(F1S // P):
            wt = wst.tile([P, HD2], F32, tag="wst", name=f"w2_t{half}_{k}")
            nc.sync.dma_start(
                out=wt[:],
                in_=t["w2_s"][k * P:(k + 1) * P, half * HD2:(half + 1) * HD2])
            for n in range(HD2 // 512):
                nc.tensor.matmul(ps_h2[:, n * 512:(n + 1) * 512],
                                 _mm(gT[:, k, :], "mm2"),
                                 _mm(wt[:, n * 512:(n + 1) * 512], "mm2"),
                                 start=(k == 0), stop=(k == F1S // P - 1))
        evict(h2_nat[:, half * HD2:(half + 1) * HD2], ps_h2[:])
    nc.gpsimd.dma_start(out=t["cc_mlp_in"][:], in_=h2_nat[:])
    nc.gpsimd.collective_compute(
        "AllReduce", ALU.add, replica_groups=GROUPS,
        ins=[t["cc_mlp_in"][:].opt()], outs=[t["cc_mlp_out"][:].opt()])

    # =======================================================================
    # STEP 9: x_pool = attn_out + h + b2 ; diffusion tail (replicated)
    # =======================================================================
    hug = nat16.tile([B, D], F32, tag="nat16", name="hug")
    nc.sync.dma_start(out=hug[:], in_=t["cc_mlp_out"][:])
    b2_bc = bcp.tile([B, D], F32, tag="bc", name="b2_bc")
    nc.gpsimd.dma_start(out=b2_bc[:], in_=_bcast(t["b2_r"][:], B))
    nc.vector.tensor_add(out=hug[:], in0=hug[:], in1=b2_bc[:])
    nc.vector.tensor_add(out=attn_nat[:], in0=attn_nat[:], in1=hug[:])
    xpT = singles.tile([P, DC, B], F32)
    t_nat_to_T(attn_nat, xpT, DC, B, "xp")

    # ---- cond (fourier features + tiny mlp), all batches ----
    # rows 0-15: cos = sin(2*pi*v + pi/2); rows 16-31: sin. One activation
    # with a per-partition phase bias (partition-offset writes must be
    # 32-aligned, so the two halves cannot be written separately).
    fw_sb = singles.tile([TD, 1], F32)
    nc.sync.dma_start(out=fw_sb[:], in_=t["four_w2"][:])
    ph_sb = singles.tile([TD, 1], F32)
    nc.sync.dma_start(out=ph_sb[:], in_=t["phase2"][:])
    tb32 = singles.tile([TD, B], F32)
    nc.gpsimd.dma_start(out=tb32[:], in_=_bcast(t["timeT"][:], TD))
    fu = singles.tile([TD, B], F32)
    nc.vector.tensor_scalar_mul(out=fu[:], in0=tb32[:], scalar1=fw_sb[:])
    # exact range reduction: sin/cos have period 1 in fu, so subtract the
    # integer part via an f32->i32->f32 round-trip (|fu| < ~64 here).
    fi = singles.tile([TD, B], mybir.dt.int32)
    nc.vector.tensor_copy(out=fi[:], in_=fu[:])
    fif = singles.tile([TD, B], F32)
    nc.vector.tensor_copy(out=fif[:], in_=fi[:])
    nc.vector.tensor_sub(out=fu[:], in0=fu[:], in1=fif[:])
    ffT = singles.tile([TD, B], F32)
    nc.scalar.activation(out=ffT[:], in_=fu[:], func=AF.Sin,
                         scale=TWO_PI, bias=ph_sb[:])
    cw1_sb = singles.tile([TD, 2 * TD], F32)
    nc.sync.dma_start(out=cw1_sb[:], in_=t["cond_w1"][:])
    cb1_sb = singles.tile([2 * TD, 1], F32)
    nc.sync.dma_start(out=cb1_sb[:], in_=t["cond_b1c"][:])
    cw2_sb = singles.tile([2 * TD, TD], F32)
    nc.sync.dma_start(out=cw2_sb[:], in_=t["cond_w2"][:])
    cb2_sb = singles.tile([TD, 1], F32)
    nc.sync.dma_start(out=cb2_sb[:], in_=t["cond_b2c"][:])

    ps_c1 = psB.tile([P, 8], F32, tag="tp8", name="ps_c1")
    nc.tensor.matmul(ps_c1[:2 * TD, :B], cw1_sb[:], ffT[:], start=True, stop=True)
    c1 = singles.tile([2 * TD, B], F32)
    nc.scalar.activation(out=c1[:], in_=ps_c1[:2 * TD, :B], func=AF.Silu,
                         bias=cb1_sb[:])
    ps_c2 = psB.tile([P, 8], F32, tag="tp8", name="ps_c2")
    nc.tensor.matmul(ps_c2[:TD, :B], cw2_sb[:], c1[:], start=True, stop=True)
    condT = singles.tile([TD, B], F32)
    nc.scalar.activation(out=condT[:], in_=ps_c2[:TD, :B], func=AF.Identity,
                         bias=cb2_sb[:])

    naT_sb = singles.tile([AD, B], F32)
    nc.sync.dma_start(out=naT_sb[:], in_=t["naT"][:])

    # ---- x0 (8, 256) = cond@rin_cond + x_pool@rin_pool + na@rin_na + rin_b
    ps_x0 = psA.tile([B, HID], F32, tag="big", name="ps_x0")
    for k in range(DC):
        wt = wst.tile([P, HID], F32, tag="wst", name=f"rp_t{k}")
        nc.sync.dma_start(out=wt[:], in_=t["rin_pool"][k * P:(k + 1) * P, :])
        nc.tensor.matmul(ps_x0[:], _mm(xpT[:, k, :], "rin"), _mm(wt[:], "rin"),
                         start=(k == 0), stop=False)
    rc_sb = singles.tile([TD, HID], F32)
    nc.sync.dma_start(out=rc_sb[:], in_=t["rin_cond"][:])
    nc.tensor.matmul(ps_x0[:], condT[:], rc_sb[:], start=False, stop=False)
    rna_sb = singles.tile([AD, HID], F32)
    nc.sync.dma_start(out=rna_sb[:], in_=t["rin_na"][:])
    nc.tensor.matmul(ps_x0[:], naT_sb[:], rna_sb[:], start=False, stop=True)
    rb_bc = bcp.tile([B, HID], F32, tag="bcs", name="rb_bc")
    nc.gpsimd.dma_start(out=rb_bc[:], in_=_bcast(t["rin_b"][:], B))
    x_nat = singles.tile([B, HID], F32)
    nc.vector.tensor_add(out=x_nat[:], in0=ps_x0[:], in1=rb_bc[:])

    # ---- 3 residual blocks ----
    for i in range(NBLK):
        xn = singles.tile([B, HID], F32, name=f"xn_{i}")
        layernorm_nat(x_nat[:], B, HID, xn[:], f"lnb{i}")
        g_bc = bcp.tile([B, HID], F32, tag="bcs", name=f"bg_bc{i}")
        nc.gpsimd.dma_start(out=g_bc[:], in_=_bcast(t["blk_ln_g"][i:i + 1, :], B))
        b_bc = bcp.tile([B, HID], F32, tag="bcs", name=f"bb_bc{i}")
        nc.gpsimd.dma_start(out=b_bc[:], in_=_bcast(t["blk_ln_b"][i:i + 1, :], B))
        nc.vector.tensor_mul(out=xn[:], in0=xn[:], in1=g_bc[:])
        nc.vector.tensor_add(out=xn[:], in0=xn[:], in1=b_bc[:])
        xnT = singles.tile([P, HID // P, B], F32, name=f"xnT_{i}")
        t_nat_to_T(xn, xnT, HID // P, B, f"xn{i}")

        ps_bh = psA.tile([B, 4 * HID], F32, tag="big", name=f"ps_bh_{i}")
        for k in range(HID // P):
            wt = wst.tile([P, 4 * HID], F32, tag="wst", name=f"bw1_t{i}_{k}")
            nc.sync.dma_start(out=wt[:], in_=t["blk_w1"][i, k * P:(k + 1) * P, :])
            for n in range(4 * HID // 512):
                nc.tensor.matmul(ps_bh[:, n * 512:(n + 1) * 512],
                                 _mm(xnT[:, k, :], "tail"),
                                 _mm(wt[:, n * 512:(n + 1) * 512], "tail"),
                                 start=(k == 0), stop=(k == HID // P - 1))
        hb_bc = bcp.tile([B, 4 * HID], F32, tag="bcs", name=f"b1_bc{i}")
        nc.gpsimd.dma_start(out=hb_bc[:], in_=_bcast(t["blk_b1"][i:i + 1, :], B))
        hb = nat8.tile([B, 4 * HID], F32, tag="nat8", name=f"hb_{i}")
        nc.vector.tensor_add(out=hb[:], in0=ps_bh[:], in1=hb_bc[:])
        nc.scalar.activation(out=hb[:], in_=hb[:], func=AF.Silu)
        hbT = singles.tile([P, 4 * HID // P, B], F32, name=f"hbT_{i}")
        t_nat_to_T(hb, hbT, 4 * HID // P, B, f"hb{i}")

        ps_bo = psA.tile([B, HID], F32, tag="big", name=f"ps_bo_{i}")
        for k in range(4 * HID // P):
            wt = wst.tile([P, HID], F32, tag="wst", name=f"bw2_t{i}_{k}")
            nc.sync.dma_start(out=wt[:], in_=t["blk_w2"][i, k * P:(k + 1) * P, :])
            nc.tensor.matmul(ps_bo[:], _mm(hbT[:, k, :], "tail"),
                             _mm(wt[:], "tail"),
                             start=(k == 0), stop=(k == 4 * HID // P - 1))
        b2b = bcp.tile([B, HID], F32, tag="bcs", name=f"b2_bc{i}")
        nc.gpsimd.dma_start(out=b2b[:], in_=_bcast(t["blk_b2"][i:i + 1, :], B))
        nc.vector.tensor_add(out=b2b[:], in0=ps_bo[:], in1=b2b[:])
        nc.vector.tensor_add(out=x_nat[:], in0=x_nat[:], in1=b2b[:])

    # ---- final: res (7, 8) = (swish(x) @ out_w + out_b).T
    nc.scalar.activation(out=x_nat[:], in_=x_nat[:], func=AF.Silu)
    xsT = singles.tile([P, HID // P, B], F32)
    t_nat_to_T(x_nat, xsT, HID // P, B, "xs")
    ow_sb = singles.tile([P, HID // P, AD], F32)
    nc.sync.dma_start(out=ow_sb[:],
                      in_=t["out_w"][:].rearrange("(c p) a -> p c a", p=P))
    ob_bc = singles.tile([B, AD], F32)
    nc.gpsimd.dma_start(out=ob_bc[:], in_=_bcast(t["out_bc"][:], B))
    ps_o = psB.tile([P, 8], F32, tag="tp8", name="ps_o")
    for k in range(HID // P):
        nc.tensor.matmul(ps_o[:B, :AD], _mm(xsT[:, k, :], "tail"),
                         _mm(ow_sb[:, k, :], "tail"),
                         start=(k == 0), stop=(k == HID // P - 1))
    out_sb = singles.tile([B, AD], F32)
    nc.vector.tensor_add(out=out_sb[:], in0=ps_o[:B, :AD], in1=ob_bc[:])
    nc.sync.dma_start(out=t["res"][:], in_=out_sb[:])


_CACHED_NC = None


def _get_nc():
    global _CACHED_NC
    if _CACHED_NC is None:
        _CACHED_NC = build_program()
    return _CACHED_NC


def _prep_in_maps(inputs):
    f32 = np.float32
    llm_full = np.ascontiguousarray(np.asarray(inputs["llm_output"], dtype=f32))
    wq = np.asarray(inputs["wq"], f32); wk = np.asarray(inputs["wk"], f32)
    wv = np.asarray(inputs["wv"], f32); wo = np.asarray(inputs["wo"], f32)
    bq = np.asarray(inputs["bq"], f32); bv = np.asarray(inputs["bv"], f32)
    bo = np.asarray(inputs["bo"], f32)
    w1 = np.asarray(inputs["mlp_w1"], f32); b1 = np.asarray(inputs["mlp_b1"], f32)
    w2 = np.asarray(inputs["mlp_w2"], f32); b2 = np.asarray(inputs["mlp_b2"], f32)
    rin_w = np.asarray(inputs["rin_w"], f32)
    probe = np.asarray(inputs["probe"], f32).reshape(D)

    def r128(v):  # (n*128,) -> (128, n) partition-major
        return np.ascontiguousarray(v.reshape(-1, P).T)

    shared = {
        "bo_r": bo.reshape(1, D),
        "ln_g_r": r128(np.asarray(inputs["ln_g"], f32)),
        "ln_b_r": r128(np.asarray(inputs["ln_b"], f32)),
        "b2_r": b2.reshape(1, D),
        "probe_r": r128(probe),
        "four_w2": np.concatenate(
            [np.asarray(inputs["four_w"], f32).reshape(TD // 2, 1)] * 2),
        "phase2": np.concatenate(
            [np.full((TD // 2, 1), np.pi / 2, f32),
             np.zeros((TD // 2, 1), f32)]),
        "timeT": np.ascontiguousarray(np.asarray(inputs["time"], f32).T),
        "naT": np.ascontiguousarray(np.asarray(inputs["noisy_actions"], f32).T),
        "cond_w1": np.asarray(inputs["cond_w1"], f32),
        "cond_b1c": np.asarray(inputs["cond_b1"], f32).reshape(-1, 1),
        "cond_w2": np.asarray(inputs["cond_w2"], f32),
        "cond_b2c": np.asarray(inputs["cond_b2"], f32).reshape(-1, 1),
        "rin_cond": np.ascontiguousarray(rin_w[0:TD]),
        "rin_pool": np.ascontiguousarray(rin_w[TD:TD + D]),
        "rin_na": np.ascontiguousarray(rin_w[TD + D:]),
        "rin_b": np.asarray(inputs["rin_b"], f32).reshape(1, HID),
        "blk_ln_g": np.asarray(inputs["blk_ln_g"], f32),
        "blk_ln_b": np.asarray(inputs["blk_ln_b"], f32),
        "blk_w1": np.asarray(inputs["blk_w1"], f32),
        "blk_b1": np.asarray(inputs["blk_b1"], f32),
        "blk_w2": np.asarray(inputs["blk_w2"], f32),
        "blk_b2": np.asarray(inputs["blk_b2"], f32),
        "out_w": np.asarray(inputs["out_w"], f32),
        "out_bc": np.asarray(inputs["out_b"], f32).reshape(1, AD),
    }

    in_maps = []
    for i in range(NC):
        hb = slice(i * DH, (i + 1) * DH)
        fb = slice(i * F1S, (i + 1) * F1S)
        m = dict(shared)
        m["llm"] = llm_full[i]
        m["llmT"] = np.ascontiguousarray(llm_full[i].T).astype(ml_dtypes.bfloat16)
        m["wq_s"] = np.ascontiguousarray(wq[:, hb])
        m["bq_s"] = np.ascontiguousarray(bq[hb]).reshape(1, DH)
        m["wkT_s"] = np.ascontiguousarray(wk[:, hb].T)
        m["wv_s"] = np.ascontiguousarray(wv[:, hb])
        m["bv_s"] = np.ascontiguousarray(bv[hb]).reshape(1, DH)
        m["wo_s"] = np.ascontiguousarray(wo[hb, :])
        m["w1_s"] = np.ascontiguousarray(w1[:, fb])
        m["b1_s"] = np.ascontiguousarray(b1[fb]).reshape(1, F1S)
        m["w2_s"] = np.ascontiguousarray(w2[fb, :])
        in_maps.append(m)
    return in_maps


def kernel(**inputs):
    nc = _get_nc()
    in_maps = _prep_in_maps(inputs)
    r = run_bass_kernel_spmd(nc, in_maps, core_ids=list(range(NC)))
    return np.ascontiguousarray(r.results[0]["res"]).astype(np.float32)


def run_traced(**inputs):
    """Like kernel() but with NTFF tracing; returns (output, results)."""
    nc = _get_nc()
    in_maps = _prep_in_maps(inputs)
    r = run_bass_kernel_spmd(nc, in_maps, core_ids=list(range(NC)), trace=True)
    return np.ascontiguousarray(r.results[0]["res"]).astype(np.float32), r


# revision 21
# speedup vs baseline: 1.0863x; 1.0863x over previous
"""Trainium2 Bass kernel for nn_DiffusionActionHead (B=8, S=2048, D=4096).

Strategy (8 NeuronCores):
  - Data-parallel over batch for everything touching llm_output (32 MiB/core).
  - Tensor-parallel weight reads: core i reads column-slice i of wq/wk/wv,
    row-slice i of wo, column/row slice i of mlp_w1/mlp_w2 (~96 MiB of
    weights split 8 ways), tiny diffusion tail replicated.
  - MAP-head attention with q_len=1 is collapsed algebraically:
        scores[s,h] = llm[s,:] . U[:,h],   U[:,h] = wk[:,hb] @ q_h / sqrt(DH)
        pooled[h,:] = softmax(scores)[h,:] @ llm
        ctx[hb]     = pooled[h,:] @ wv[:,hb] + bv[hb]
    (bk shifts scores by a per-head constant -> cancels in softmax.)
  - 4 small collectives: AllGather(U cols), AllToAll(pooled, head<->batch),
    AllReduce(attn_out partial), AllReduce(mlp partial).
Activations are kept feature-on-partition ("transposed") so every big matmul
streams the weight slice in its natural DRAM layout as the moving operand.
"""

import numpy as np
import ml_dtypes
import sys

if "/opt/trn_rl_repo" not in sys.path:
    sys.path.insert(0, "/opt/trn_rl_repo")

import concourse.bass as bass
import concourse.tile as tile
from concourse import bacc, mybir
from concourse.masks import make_identity
from concourse.bass_utils import run_bass_kernel_spmd

F32 = mybir.dt.float32
F32R = mybir.dt.float32r
BF16 = mybir.dt.bfloat16
AF = mybir.ActivationFunctionType
ALU = mybir.AluOpType

B, S, D = 8, 2048, 4096
H, AD, TD, HID, NBLK = 8, 7, 32, 256, 3
DH = D // H            # 512
NC = 8                 # cores
P = 128
SC = S // P            # 16 S-chunks
DC = D // P            # 32 D-chunks
HD2 = D // 2           # 2048 (half width for 4-bank PSUM tiles)
F1S = 4 * D // NC      # 2048 per-core hidden cols of mlp_w1
RSQRT_DH = 1.0 / float(np.sqrt(DH))
TWO_PI = 2.0 * float(np.pi)

# matmul dtype knob per family: "f32" (exact, 4 cyc/row) or "f32r" (fast).
MM_KNOB = {
    "q": "f32r", "u": "f32r", "pooled": "f32r", "ctx": "f32r",
    "attn": "f32r", "mm1": "f32r", "mm2": "f32r", "rin": "f32r",
    "tail": "f32",
}


def _mm(ap, fam):
    if MM_KNOB[fam] == "f32r" and ap.dtype == F32:
        return ap.bitcast(F32R)
    return ap


def _bcast(src_ap, nparts):
    """Partition-broadcast a (1, N) DRAM AP to (nparts, N)."""
    ap = src_ap
    assert ap.shape[0] == 1, ap.shape
    return bass.AP(tensor=ap.tensor, offset=ap.offset,
                   ap=[[0, nparts]] + [list(x) for x in ap.ap[1:]])


def build_program():
    nc = bacc.Bacc("TRN2", target_bir_lowering=False, debug=False,
                   num_devices=NC)

    t = {}

    def din(name, shape, dtype=F32):
        t[name] = nc.dram_tensor(name, shape, dtype, kind="ExternalInput")

    din("llm", [S, D]); din("llmT", [D, S], BF16)
    din("wq_s", [D, DH]); din("bq_s", [1, DH])
    din("wkT_s", [DH, D])
    din("wv_s", [D, DH]); din("bv_s", [1, DH])
    din("wo_s", [DH, D]); din("bo_r", [1, D])
    din("ln_g_r", [P, DC]); din("ln_b_r", [P, DC])
    din("w1_s", [D, F1S]); din("b1_s", [1, F1S])
    din("w2_s", [F1S, D]); din("b2_r", [1, D])
    din("probe_r", [P, DC])
    din("four_w2", [TD, 1]); din("phase2", [TD, 1])
    din("timeT", [1, B]); din("naT", [AD, B])
    din("cond_w1", [TD, 2 * TD]); din("cond_b1c", [2 * TD, 1])
    din("cond_w2", [2 * TD, TD]); din("cond_b2c", [TD, 1])
    din("rin_cond", [TD, HID]); din("rin_pool", [D, HID])
    din("rin_na", [AD, HID]); din("rin_b", [1, HID])
    din("blk_ln_g", [NBLK, HID]); din("blk_ln_b", [NBLK, HID])
    din("blk_w1", [NBLK, HID, 4 * HID]); din("blk_b1", [NBLK, 4 * HID])
    din("blk_w2", [NBLK, 4 * HID, HID]); din("blk_b2", [NBLK, HID])
    din("out_w", [HID, AD]); din("out_bc", [1, AD])
    t["res"] = nc.dram_tensor("res", [B, AD], F32, kind="ExternalOutput")

    # collective bounce buffers (internal DRAM; outputs in Shared space)
    t["cc_u_in"] = nc.dram_tensor("cc_u_in", [1, D], F32)
    t["cc_u_out"] = nc.dram_tensor("cc_u_out", [NC, D], F32, addr_space="Shared")
    t["cc_pool_in"] = nc.dram_tensor("cc_pool_in", [H, D], F32)
    t["cc_pool_out"] = nc.dram_tensor("cc_pool_out", [B, D], F32)
    t["cc_attn_in"] = nc.dram_tensor("cc_attn_in", [B, D], F32)
    t["cc_attn_out"] = nc.dram_tensor("cc_attn_out", [B, D], F32,
                                      addr_space="Shared")
    t["cc_mlp_in"] = nc.dram_tensor("cc_mlp_in", [B, D], F32)
    t["cc_mlp_out"] = nc.dram_tensor("cc_mlp_out", [B, D], F32,
                                     addr_space="Shared")

    with tile.TileContext(nc) as tc:
        import contextlib
        with contextlib.ExitStack() as ctx:
            _build(nc, tc, t, ctx)
    nc.finalize()
    return nc


def _build(nc, tc, t, ctx):
    GROUPS = [list(range(NC))]

    singles = ctx.enter_context(tc.tile_pool(name="singles", bufs=1))
    llm_pool = ctx.enter_context(tc.tile_pool(name="llm_pool", bufs=3))
    llmT_pool = ctx.enter_context(tc.tile_pool(name="llmT_pool", bufs=2))
    wst = ctx.enter_context(tc.tile_pool(name="wst", bufs=4))
    nat16 = ctx.enter_context(tc.tile_pool(name="nat16", bufs=2))
    nat8 = ctx.enter_context(tc.tile_pool(name="nat8", bufs=2))
    bcp = ctx.enter_context(tc.tile_pool(name="bcp", bufs=1))
    psA = ctx.enter_context(tc.tile_pool(name="psA", bufs=1, space="PSUM"))
    psB = ctx.enter_context(tc.tile_pool(name="psB", bufs=2, space="PSUM"))
    psC = ctx.enter_context(tc.tile_pool(name="psC", bufs=2, space="PSUM"))

    ident = singles.tile([P, P], F32)
    make_identity(nc, ident)
    eps_sb = singles.tile([P, 1], F32)
    nc.vector.memset(eps_sb[:], 1e-5)

    def evict(dst, src):
        nc.vector.tensor_copy(out=dst, in_=src)

    def t_nat_to_T(src_nat, dst_T, nchunks, npart, uid):
        """(npart, nchunks*128) sbuf -> (128, nchunks, npart) sbuf via PE."""
        for c in range(nchunks):
            ps = psB.tile([P, 8], F32, tag="tp8", name=f"tp_{uid}_{c}")
            nc.tensor.transpose(ps[:, :npart], src_nat[:, c * P:(c + 1) * P],
                                ident[:npart, :npart])
            evict(dst_T[:, c, :], ps[:, :npart])

    def layernorm_nat(x_nat, npart, n, y_nat, uid, eps=1e-5):
        """y = (x - mean) / sqrt(var + eps) over free dim of (npart, n)."""
        nsub = max(1, n // 512)
        st = nat8.tile([npart, nsub, nc.vector.BN_STATS_DIM], F32, tag="lnst",
                       name=f"lnst_{uid}")
        xg = x_nat.rearrange("p (a b) -> p a b", a=nsub)
        for g in range(nsub):
            nc.vector.bn_stats(out=st[:, g, :], in_=xg[:, g, :])
        mv = nat8.tile([npart, nc.vector.BN_AGGR_DIM], F32, tag="lnmv",
                       name=f"lnmv_{uid}")
        nc.vector.bn_aggr(out=mv[:], in_=st[:])
        std = nat8.tile([npart, 1], F32, tag="lnsd", name=f"lnsd_{uid}")
        nc.scalar.activation(out=std[:], in_=mv[:, 1:2], func=AF.Sqrt,
                             bias=eps_sb[:npart, :])
        nc.vector.reciprocal(out=std[:], in_=std[:])
        nc.vector.tensor_scalar(out=y_nat, in0=x_nat, scalar1=mv[:, 0:1],
                                scalar2=std[:], op0=ALU.subtract, op1=ALU.mult)

    # =======================================================================
    # STEP 1: q_s = (probe @ wq_s + bq_s) / sqrt(DH)   -> (1, 512) natural
    # =======================================================================
    probe_sb = singles.tile([P, DC], F32R)
    nc.sync.dma_start(out=probe_sb[:], in_=t["probe_r"][:].bitcast(F32R))
    bq_sb = singles.tile([1, DH], F32)
    nc.sync.dma_start(out=bq_sb[:], in_=t["bq_s"][:])

    q_nat = singles.tile([1, DH], F32)
    ps_q = psC.tile([1, DH], F32, tag="vec", name="ps_q")
    for k in range(DC):
        wt = wst.tile([P, HD2], F32R, tag="wst", name=f"wq_t{k}")
        nc.sync.dma_start(out=wt[:, :DH],
                          in_=t["wq_s"][k * P:(k + 1) * P, :].bitcast(F32R))
        nc.tensor.matmul(ps_q[:], _mm(probe_sb[:, k:k + 1], "q"),
                         _mm(wt[:, :DH], "q"),
                         start=(k == 0), stop=(k == DC - 1))
    nc.vector.tensor_add(out=q_nat[:], in0=ps_q[:], in1=bq_sb[:])
    nc.vector.tensor_scalar_mul(out=q_nat[:], in0=q_nat[:], scalar1=RSQRT_DH)

    qT = singles.tile([P, DH // P], F32R)  # (128, 4)
    for c in range(DH // P):
        ps = psB.tile([P, 8], F32, tag="tp8", name=f"tp_q_{c}")
        nc.tensor.transpose(ps[:, :1], q_nat[:, c * P:(c + 1) * P], ident[:1, :1])
        evict(qT[:, c:c + 1], ps[:, :1])

    # =======================================================================
    # STEP 2: U column of this core's head: U = wkT_s.T @ q~  -> (1, 4096)
    #         AllGather -> cc_u_out (8, 4096) = U.T with one row per head
    # =======================================================================
    u_nat = singles.tile([1, D], F32)
    for nhalf in range(2):
        wk_tiles = []
        for k in range(DH // P):
            wt = wst.tile([P, HD2], F32R, tag="wst", name=f"wk_t{nhalf}_{k}")
            nc.sync.dma_start(
                out=wt[:],
                in_=t["wkT_s"][k * P:(k + 1) * P,
                               nhalf * HD2:(nhalf + 1) * HD2].bitcast(F32R))
            wk_tiles.append(wt)
        for ncol in range(4):
            n0 = nhalf * 4 + ncol
            ps_u = psC.tile([1, DH], F32, tag="vec", name=f"ps_u_{n0}")
            for k in range(DH // P):
                nc.tensor.matmul(
                    ps_u[:], _mm(qT[:, k:k + 1], "u"),
                    _mm(wk_tiles[k][:, ncol * DH:(ncol + 1) * DH], "u"),
                    start=(k == 0), stop=(k == DH // P - 1))
            evict(u_nat[:, n0 * DH:(n0 + 1) * DH], ps_u[:])

    nc.gpsimd.dma_start(out=t["cc_u_in"][:], in_=u_nat[:])
    nc.gpsimd.collective_compute(
        "AllGather", ALU.bypass, replica_groups=GROUPS,
        ins=[t["cc_u_in"][:].opt()], outs=[t["cc_u_out"][:].opt()])

    # read back U.T (8, 4096), transpose to (128, 32, 8), cast to bf16
    uh_nat = nat16.tile([H, D], F32, tag="nat16", name="uh_nat")
    nc.sync.dma_start(out=uh_nat[:], in_=t["cc_u_out"][:])
    u_bf = singles.tile([P, DC, H], BF16)
    for c in range(DC):
        ps = psB.tile([P, 8], F32, tag="tp8", name=f"tp_u_{c}")
        nc.tensor.transpose(ps[:, :H], uh_nat[:, c * P:(c + 1) * P],
                            ident[:H, :H])
        evict(u_bf[:, c, :], ps[:, :H])

    # =======================================================================
    # STEP 3: scoresT (8, 2048) = U.T @ llmT  (bf16 inputs, fp32 accum)
    # =======================================================================
    ps_sc = psA.tile([H, S], F32, tag="big", name="ps_sc")
    for k in range(DC):
        lt = llmT_pool.tile([P, S], BF16, tag="llmT", name=f"llmT_t{k}")
        nc.sync.dma_start(out=lt[:], in_=t["llmT"][k * P:(k + 1) * P, :])
        for n in range(S // 512):
            nc.tensor.matmul(ps_sc[:, n * 512:(n + 1) * 512],
                             u_bf[:, k, :], lt[:, n * 512:(n + 1) * 512],
                             start=(k == 0), stop=(k == DC - 1))

    # =======================================================================
    # STEP 4: softmax over S. Max-subtraction is skipped deliberately:
    # softmax is shift-invariant and |scores| here is < ~1, so exp() is
    # perfectly conditioned; result is mathematically identical.
    # =======================================================================
    p_nat = nat8.tile([H, S], F32, tag="nat8", name="p_nat")
    nc.scalar.activation(out=p_nat[:], in_=ps_sc[:], func=AF.Exp)
    den = singles.tile([H, 1], F32)
    nc.vector.reduce_sum(out=den[:], in_=p_nat[:], axis=mybir.AxisListType.X)
    nc.vector.reciprocal(out=den[:], in_=den[:])
    nc.vector.tensor_scalar_mul(out=p_nat[:], in0=p_nat[:], scalar1=den[:])
    pT = singles.tile([P, SC, H], F32R)
    t_nat_to_T(p_nat, pT, SC, H, "p")

    # =======================================================================
    # STEP 5: pooled (8, 4096) = pT.T @ llm ; AllToAll (head <-> batch)
    # =======================================================================
    pooled_nat = nat16.tile([H, D], F32, tag="nat16", name="pooled_nat")
    for half in range(2):
        ps_p = psA.tile([H, HD2], F32, tag="big", name=f"ps_pool_{half}")
        for s in range(SC):
            lt = llm_pool.tile([P, HD2], F32R, tag="llm", name=f"llm_t{half}_{s}")
            nc.sync.dma_start(
                out=lt[:],
                in_=t["llm"][s * P:(s + 1) * P,
                             half * HD2:(half + 1) * HD2].bitcast(F32R))
            for n in range(HD2 // 512):
                nc.tensor.matmul(ps_p[:, n * 512:(n + 1) * 512],
                                 _mm(pT[:, s, :], "pooled"),
                                 _mm(lt[:, n * 512:(n + 1) * 512], "pooled"),
                                 start=(s == 0), stop=(s == SC - 1))
        evict(pooled_nat[:, half * HD2:(half + 1) * HD2], ps_p[:])

    nc.gpsimd.dma_start(out=t["cc_pool_in"][:], in_=pooled_nat[:])
    nc.gpsimd.collective_compute(
        "AllToAll", ALU.bypass, replica_groups=GROUPS,
        ins=[t["cc_pool_in"][:].opt()], outs=[t["cc_pool_out"][:].opt()])

    # =======================================================================
    # STEP 6: ctx for this core's head, all batches: (8, 512)
    # =======================================================================
    poolh_nat = nat16.tile([B, D], F32, tag="nat16", name="poolh_nat")
    nc.sync.dma_start(out=poolh_nat[:], in_=t["cc_pool_out"][:])
    poolhT = singles.tile([P, DC, B], F32R)
    t_nat_to_T(poolh_nat, poolhT, DC, B, "ph")

    bv_bc = bcp.tile([B, D], F32, tag="bc", name="bv_bc")
    nc.gpsimd.dma_start(out=bv_bc[:, :DH], in_=_bcast(t["bv_s"][:], B))
    ps_cx = psA.tile([B, DH], F32, tag="big", name="ps_cx")
    for k in range(DC):
        wt = wst.tile([P, HD2], F32R, tag="wst", name=f"wv_t{k}")
        nc.sync.dma_start(out=wt[:, :DH],
                          in_=t["wv_s"][k * P:(k + 1) * P, :].bitcast(F32R))
        nc.tensor.matmul(ps_cx[:], _mm(poolhT[:, k, :], "ctx"),
                         _mm(wt[:, :DH], "ctx"),
                         start=(k == 0), stop=(k == DC - 1))
    ctx_nat = nat8.tile([B, DH], F32, tag="nat8", name="ctx_nat")
    nc.vector.tensor_add(out=ctx_nat[:], in0=ps_cx[:], in1=bv_bc[:, :DH])
    ctxT = singles.tile([P, DH // P, B], F32R)
    t_nat_to_T(ctx_nat, ctxT, DH // P, B, "cx")

    # =======================================================================
    # STEP 7: attn_out partial (8, 4096) = ctx @ wo_s ; AllReduce
    # =======================================================================
    attn_part = nat16.tile([B, D], F32, tag="nat16", name="attn_part")
    for half in range(2):
        ps_a = psA.tile([B, HD2], F32, tag="big", name=f"ps_attn_{half}")
        for k in range(DH // P):
            wt = wst.tile([P, HD2], F32R, tag="wst", name=f"wo_t{half}_{k}")
            nc.sync.dma_start(
                out=wt[:],
                in_=t["wo_s"][k * P:(k + 1) * P,
                              half * HD2:(half + 1) * HD2].bitcast(F32R))
            for n in range(HD2 // 512):
                nc.tensor.matmul(ps_a[:, n * 512:(n + 1) * 512],
                                 _mm(ctxT[:, k, :], "attn"),
                                 _mm(wt[:, n * 512:(n + 1) * 512], "attn"),
                                 start=(k == 0), stop=(k == DH // P - 1))
        evict(attn_part[:, half * HD2:(half + 1) * HD2], ps_a[:])
    nc.gpsimd.dma_start(out=t["cc_attn_in"][:], in_=attn_part[:])
    nc.gpsimd.collective_compute(
        "AllReduce", ALU.add, replica_groups=GROUPS,
        ins=[t["cc_attn_in"][:].opt()], outs=[t["cc_attn_out"][:].opt()])

    # =======================================================================
    # STEP 8: attn_out = AR + bo ; y = LN(attn_out)*g+b ; mlp partial ; AR
    # =======================================================================
    attn_nat = singles.tile([B, D], F32)  # persists (residual)
    nc.sync.dma_start(out=attn_nat[:], in_=t["cc_attn_out"][:])
    bo_bc = bcp.tile([B, D], F32, tag="bc", name="bo_bc")
    nc.gpsimd.dma_start(out=bo_bc[:], in_=_bcast(t["bo_r"][:], B))
    nc.vector.tensor_add(out=attn_nat[:], in0=attn_nat[:], in1=bo_bc[:])

    y_nat = nat16.tile([B, D], F32, tag="nat16", name="y_nat")
    layernorm_nat(attn_nat[:], B, D, y_nat[:], "ln0")
    yT = singles.tile([P, DC, B], F32R)
    t_nat_to_T(y_nat, yT, DC, B, "y")
    # LN affine in T layout (gamma/beta become per-partition scalars)
    lng_sb = singles.tile([P, DC], F32)
    nc.sync.dma_start(out=lng_sb[:], in_=t["ln_g_r"][:])
    lnb_sb = singles.tile([P, DC], F32)
    nc.sync.dma_start(out=lnb_sb[:], in_=t["ln_b_r"][:])
    for c in range(DC):
        nc.vector.tensor_scalar(out=yT[:, c, :], in0=yT[:, c, :],
                                scalar1=lng_sb[:, c:c + 1],
                                scalar2=lnb_sb[:, c:c + 1],
                                op0=ALU.mult, op1=ALU.add)

    # mm1: h1 (8, 2048) = y @ w1_s ; + b1 ; exact gelu
    ps_h1 = psA.tile([B, F1S], F32, tag="big", name="ps_h1")
    for k in range(DC):
        wt = wst.tile([P, F1S], F32R, tag="wst", name=f"w1_t{k}")
        nc.sync.dma_start(out=wt[:],
                          in_=t["w1_s"][k * P:(k + 1) * P, :].bitcast(F32R))
        for n in range(F1S // 512):
            nc.tensor.matmul(ps_h1[:, n * 512:(n + 1) * 512],
                             _mm(yT[:, k, :], "mm1"),
                             _mm(wt[:, n * 512:(n + 1) * 512], "mm1"),
                             start=(k == 0), stop=(k == DC - 1))
    b1_bc = bcp.tile([B, F1S], F32, tag="bc2", name="b1_bc")
    nc.gpsimd.dma_start(out=b1_bc[:], in_=_bcast(t["b1_s"][:], B))
    g_nat = nat8.tile([B, F1S], F32, tag="nat8", name="g_nat")
    nc.vector.tensor_add(out=g_nat[:], in0=ps_h1[:], in1=b1_bc[:])
    nc.scalar.activation(out=g_nat[:], in_=g_nat[:], func=AF.Gelu)
    gT = singles.tile([P, F1S // P, B], F32R)
    t_nat_to_T(g_nat, gT, F1S // P, B, "g")

    # mm2: h2 partial (8, 4096) = g @ w2_s ; AllReduce
    h2_nat = nat16.tile([B, D], F32, tag="nat16", name="h2_nat")
    for half in range(2):
        ps_h2 = psA.tile([B, HD2], F32, tag="big", name=f"ps_h2_{half}")
        for k in range(F1S // P):
            wt = wst.tile([P, HD2], F32R, tag="wst", name=f"w2_t{half}_{k}")
            nc.sync.dma_start(
                out=wt[:],
                in_=t["w2_s"][k * P:(k + 1) * P,
                              half * HD2:(half + 1) * HD2].bitcast(F32R))
            for n in range(HD2 // 512):
                nc.tensor.matmul(ps_h2[:, n * 512:(n + 1) * 512],
                                 _mm(gT[:, k, :], "mm2"),
                                 _mm(wt[:, n * 512:(n + 1) * 512], "mm2"),
                                 start=(k == 0), stop=(k == F1S // P - 1))
        evict(h2_nat[:, half * HD2:(half + 1) * HD2], ps_h2[:])
    nc.gpsimd.dma_start(out=t["cc_mlp_in"][:], in_=h2_nat[:])
    nc.gpsimd.collective_compute(
        "AllReduce", ALU.add, replica_groups=GROUPS,
        ins=[t["cc_mlp_in"][:].opt()], outs=[t["cc_mlp_out"][:].opt()])

    # =======================================================================
    # STEP 9: x_pool = attn_out + h + b2 ; diffusion tail (replicated)
    # =======================================================================
    hug = nat16.tile([B, D], F32, tag="nat16", name="hug")
    nc.sync.dma_start(out=hug[:], in_=t["cc_mlp_out"][:])
    b2_bc = bcp.tile([B, D], F32, tag="bc", name="b2_bc")
    nc.gpsimd.dma_start(out=b2_bc[:], in_=_bcast(t["b2_r"][:], B))
    nc.vector.tensor_add(out=hug[:], in0=hug[:], in1=b2_bc[:])
    nc.vector.tensor_add(out=attn_nat[:], in0=attn_nat[:], in1=hug[:])
    xpT = singles.tile([P, DC, B], F32R)
    t_nat_to_T(attn_nat, xpT, DC, B, "xp")

    # ---- cond (fourier features + tiny mlp), all batches ----
    # rows 0-15: cos = sin(2*pi*v + pi/2); rows 16-31: sin. One activation
    # with a per-partition phase bias (partition-offset writes must be
    # 32-aligned, so the two halves cannot be written separately).
    fw_sb = singles.tile([TD, 1], F32)
    nc.sync.dma_start(out=fw_sb[:], in_=t["four_w2"][:])
    ph_sb = singles.tile([TD, 1], F32)
    nc.sync.dma_start(out=ph_sb[:], in_=t["phase2"][:])
    tb32 = singles.tile([TD, B], F32)
    nc.gpsimd.dma_start(out=tb32[:], in_=_bcast(t["timeT"][:], TD))
    fu = singles.tile([TD, B], F32)
    nc.vector.tensor_scalar_mul(out=fu[:], in0=tb32[:], scalar1=fw_sb[:])
    # exact range reduction: sin/cos have period 1 in fu, so subtract the
    # integer part via an f32->i32->f32 round-trip (|fu| < ~64 here).
    fi = singles.tile([TD, B], mybir.dt.int32)
    nc.vector.tensor_copy(out=fi[:], in_=fu[:])
    fif = singles.tile([TD, B], F32)
    nc.vector.tensor_copy(out=fif[:], in_=fi[:])
    nc.vector.tensor_sub(out=fu[:], in0=fu[:], in1=fif[:])
    ffT = singles.tile([TD, B], F32)
    nc.scalar.activation(out=ffT[:], in_=fu[:], func=AF.Sin,
                         scale=TWO_PI, bias=ph_sb[:])
    cw1_sb = singles.tile([TD, 2 * TD], F32)
    nc.sync.dma_start(out=cw1_sb[:], in_=t["cond_w1"][:])
    cb1_sb = singles.tile([2 * TD, 1], F32)
    nc.sync.dma_start(out=cb1_sb[:], in_=t["cond_b1c"][:])
    cw2_sb = singles.tile([2 * TD, TD], F32)
    nc.sync.dma_start(out=cw2_sb[:], in_=t["cond_w2"][:])
    cb2_sb = singles.tile([TD, 1], F32)
    nc.sync.dma_start(out=cb2_sb[:], in_=t["cond_b2c"][:])

    ps_c1 = psB.tile([P, 8], F32, tag="tp8", name="ps_c1")
    nc.tensor.matmul(ps_c1[:2 * TD, :B], cw1_sb[:], ffT[:], start=True, stop=True)
    c1 = singles.tile([2 * TD, B], F32)
    nc.scalar.activation(out=c1[:], in_=ps_c1[:2 * TD, :B], func=AF.Silu,
                         bias=cb1_sb[:])
    ps_c2 = psB.tile([P, 8], F32, tag="tp8", name="ps_c2")
    nc.tensor.matmul(ps_c2[:TD, :B], cw2_sb[:], c1[:], start=True, stop=True)
    condT = singles.tile([TD, B], F32)
    nc.scalar.activation(out=condT[:], in_=ps_c2[:TD, :B], func=AF.Identity,
                         bias=cb2_sb[:])

    naT_sb = singles.tile([AD, B], F32)
    nc.sync.dma_start(out=naT_sb[:], in_=t["naT"][:])

    # ---- x0 (8, 256) = cond@rin_cond + x_pool@rin_pool + na@rin_na + rin_b
    ps_x0 = psA.tile([B, HID], F32, tag="big", name="ps_x0")
    for k in range(DC):
        wt = wst.tile([P, HID], F32R, tag="wst", name=f"rp_t{k}")
        nc.sync.dma_start(out=wt[:],
                          in_=t["rin_pool"][k * P:(k + 1) * P, :].bitcast(F32R))
        nc.tensor.matmul(ps_x0[:], _mm(xpT[:, k, :], "rin"), _mm(wt[:], "rin"),
                         start=(k == 0), stop=False)
    rc_sb = singles.tile([TD, HID], F32)
    nc.sync.dma_start(out=rc_sb[:], in_=t["rin_cond"][:])
    nc.tensor.matmul(ps_x0[:], condT[:], rc_sb[:], start=False, stop=False)
    rna_sb = singles.tile([AD, HID], F32)
    nc.sync.dma_start(out=rna_sb[:], in_=t["rin_na"][:])
    nc.tensor.matmul(ps_x0[:], naT_sb[:], rna_sb[:], start=False, stop=True)
    rb_bc = bcp.tile([B, HID], F32, tag="bcs", name="rb_bc")
    nc.gpsimd.dma_start(out=rb_bc[:], in_=_bcast(t["rin_b"][:], B))
    x_nat = singles.tile([B, HID], F32)
    nc.vector.tensor_add(out=x_nat[:], in0=ps_x0[:], in1=rb_bc[:])

    # ---- 3 residual blocks ----
    for i in range(NBLK):
        xn = singles.tile([B, HID], F32, name=f"xn_{i}")
        layernorm_nat(x_nat[:], B, HID, xn[:], f"lnb{i}")
        g_bc = bcp.tile([B, HID], F32, tag="bcs", name=f"bg_bc{i}")
        nc.gpsimd.dma_start(out=g_bc[:], in_=_bcast(t["blk_ln_g"][i:i + 1, :], B))
        b_bc = bcp.tile([B, HID], F32, tag="bcs", name=f"bb_bc{i}")
        nc.gpsimd.dma_start(out=b_bc[:], in_=_bcast(t["blk_ln_b"][i:i + 1, :], B))
        nc.vector.tensor_mul(out=xn[:], in0=xn[:], in1=g_bc[:])
        nc.vector.tensor_add(out=xn[:], in0=xn[:], in1=b_bc[:])
        xnT = singles.tile([P, HID // P, B], F32, name=f"xnT_{i}")
        t_nat_to_T(xn, xnT, HID // P, B, f"xn{i}")

        ps_bh = psA.tile([B, 4 * HID], F32, tag="big", name=f"ps_bh_{i}")
        for k in range(HID // P):
            wt = wst.tile([P, 4 * HID], F32, tag="wst", name=f"bw1_t{i}_{k}")
            nc.sync.dma_start(out=wt[:], in_=t["blk_w1"][i, k * P:(k + 1) * P, :])
            for n in range(4 * HID // 512):
                nc.tensor.matmul(ps_bh[:, n * 512:(n + 1) * 512],
                                 _mm(xnT[:, k, :], "tail"),
                                 _mm(wt[:, n * 512:(n + 1) * 512], "tail"),
                                 start=(k == 0), stop=(k == HID // P - 1))
        hb_bc = bcp.tile([B, 4 * HID], F32, tag="bcs", name=f"b1_bc{i}")
        nc.gpsimd.dma_start(out=hb_bc[:], in_=_bcast(t["blk_b1"][i:i + 1, :], B))
        hb = nat8.tile([B, 4 * HID], F32, tag="nat8", name=f"hb_{i}")
        nc.vector.tensor_add(out=hb[:], in0=ps_bh[:], in1=hb_bc[:])
        nc.scalar.activation(out=hb[:], in_=hb[:], func=AF.Silu)
        hbT = singles.tile([P, 4 * HID // P, B], F32, name=f"hbT_{i}")
        t_nat_to_T(hb, hbT, 4 * HID // P, B, f"hb{i}")

        ps_bo = psA.tile([B, HID], F32, tag="big", name=f"ps_bo_{i}")
        for k in range(4 * HID // P):
            wt = wst.tile([P, HID], F32, tag="wst", name=f"bw2_t{i}_{k}")
            nc.sync.dma_start(out=wt[:], in_=t["blk_w2"][i, k * P:(k + 1) * P, :])
            nc.tensor.matmul(ps_bo[:], _mm(hbT[:, k, :], "tail"),
                             _mm(wt[:], "tail"),
                             start=(k == 0), stop=(k == 4 * HID // P - 1))
        b2b = bcp.tile([B, HID], F32, tag="bcs", name=f"b2_bc{i}")
        nc.gpsimd.dma_start(out=b2b[:], in_=_bcast(t["blk_b2"][i:i + 1, :], B))
        nc.vector.tensor_add(out=b2b[:], in0=ps_bo[:], in1=b2b[:])
        nc.vector.tensor_add(out=x_nat[:], in0=x_nat[:], in1=b2b[:])

    # ---- final: res (7, 8) = (swish(x) @ out_w + out_b).T
    nc.scalar.activation(out=x_nat[:], in_=x_nat[:], func=AF.Silu)
    xsT = singles.tile([P, HID // P, B], F32)
    t_nat_to_T(x_nat, xsT, HID // P, B, "xs")
    ow_sb = singles.tile([P, HID // P, AD], F32)
    nc.sync.dma_start(out=ow_sb[:],
                      in_=t["out_w"][:].rearrange("(c p) a -> p c a", p=P))
    ob_bc = singles.tile([B, AD], F32)
    nc.gpsimd.dma_start(out=ob_bc[:], in_=_bcast(t["out_bc"][:], B))
    ps_o = psB.tile([P, 8], F32, tag="tp8", name="ps_o")
    for k in range(HID // P):
        nc.tensor.matmul(ps_o[:B, :AD], _mm(xsT[:, k, :], "tail"),
                         _mm(ow_sb[:, k, :], "tail"),
                         start=(k == 0), stop=(k == HID // P - 1))
    out_sb = singles.tile([B, AD], F32)
    nc.vector.tensor_add(out=out_sb[:], in0=ps_o[:B, :AD], in1=ob_bc[:])
    nc.sync.dma_start(out=t["res"][:], in_=out_sb[:])


_CACHED_NC = None


def _get_nc():
    global _CACHED_NC
    if _CACHED_NC is None:
        _CACHED_NC = build_program()
    return _CACHED_NC


def _prep_in_maps(inputs):
    f32 = np.float32
    llm_full = np.ascontiguousarray(np.asarray(inputs["llm_output"], dtype=f32))
    wq = np.asarray(inputs["wq"], f32); wk = np.asarray(inputs["wk"], f32)
    wv = np.asarray(inputs["wv"], f32); wo = np.asarray(inputs["wo"], f32)
    bq = np.asarray(inputs["bq"], f32); bv = np.asarray(inputs["bv"], f32)
    bo = np.asarray(inputs["bo"], f32)
    w1 = np.asarray(inputs["mlp_w1"], f32); b1 = np.asarray(inputs["mlp_b1"], f32)
    w2 = np.asarray(inputs["mlp_w2"], f32); b2 = np.asarray(inputs["mlp_b2"], f32)
    rin_w = np.asarray(inputs["rin_w"], f32)
    probe = np.asarray(inputs["probe"], f32).reshape(D)

    def r128(v):  # (n*128,) -> (128, n) partition-major
        return np.ascontiguousarray(v.reshape(-1, P).T)

    shared = {
        "bo_r": bo.reshape(1, D),
        "ln_g_r": r128(np.asarray(inputs["ln_g"], f32)),
        "ln_b_r": r128(np.asarray(inputs["ln_b"], f32)),
        "b2_r": b2.reshape(1, D),
        "probe_r": r128(probe),
        "four_w2": np.concatenate(
            [np.asarray(inputs["four_w"], f32).reshape(TD // 2, 1)] * 2),
        "phase2": np.concatenate(
            [np.full((TD // 2, 1), np.pi / 2, f32),
             np.zeros((TD // 2, 1), f32)]),
        "timeT": np.ascontiguousarray(np.asarray(inputs["time"], f32).T),
        "naT": np.ascontiguousarray(np.asarray(inputs["noisy_actions"], f32).T),
        "cond_w1": np.asarray(inputs["cond_w1"], f32),
        "cond_b1c": np.asarray(inputs["cond_b1"], f32).reshape(-1, 1),
        "cond_w2": np.asarray(inputs["cond_w2"], f32),
        "cond_b2c": np.asarray(inputs["cond_b2"], f32).reshape(-1, 1),
        "rin_cond": np.ascontiguousarray(rin_w[0:TD]),
        "rin_pool": np.ascontiguousarray(rin_w[TD:TD + D]),
        "rin_na": np.ascontiguousarray(rin_w[TD + D:]),
        "rin_b": np.asarray(inputs["rin_b"], f32).reshape(1, HID),
        "blk_ln_g": np.asarray(inputs["blk_ln_g"], f32),
        "blk_ln_b": np.asarray(inputs["blk_ln_b"], f32),
        "blk_w1": np.asarray(inputs["blk_w1"], f32),
        "blk_b1": np.asarray(inputs["blk_b1"], f32),
        "blk_w2": np.asarray(inputs["blk_w2"], f32),
        "blk_b2": np.asarray(inputs["blk_b2"], f32),
        "out_w": np.asarray(inputs["out_w"], f32),
        "out_bc": np.asarray(inputs["out_b"], f32).reshape(1, AD),
    }

    in_maps = []
    for i in range(NC):
        hb = slice(i * DH, (i + 1) * DH)
        fb = slice(i * F1S, (i + 1) * F1S)
        m = dict(shared)
        m["llm"] = llm_full[i]
        m["llmT"] = np.ascontiguousarray(llm_full[i].T).astype(ml_dtypes.bfloat16)
        m["wq_s"] = np.ascontiguousarray(wq[:, hb])
        m["bq_s"] = np.ascontiguousarray(bq[hb]).reshape(1, DH)
        m["wkT_s"] = np.ascontiguousarray(wk[:, hb].T)
        m["wv_s"] = np.ascontiguousarray(wv[:, hb])
        m["bv_s"] = np.ascontiguousarray(bv[hb]).reshape(1, DH)
        m["wo_s"] = np.ascontiguousarray(wo[hb, :])
        m["w1_s"] = np.ascontiguousarray(w1[:, fb])
        m["b1_s"] = np.ascontiguousarray(b1[fb]).reshape(1, F1S)
        m["w2_s"] = np.ascontiguousarray(w2[fb, :])
        in_maps.append(m)
    return in_maps


def kernel(**inputs):
    nc = _get_nc()
    in_maps = _prep_in_maps(inputs)
    r = run_bass_kernel_spmd(nc, in_maps, core_ids=list(range(NC)))
    return np.ascontiguousarray(r.results[0]["res"]).astype(np.float32)


def run_traced(**inputs):
    """Like kernel() but with NTFF tracing; returns (output, results)."""
    nc = _get_nc()
    in_maps = _prep_in_maps(inputs)
    r = run_bass_kernel_spmd(nc, in_maps, core_ids=list(range(NC)), trace=True)
    return np.ascontiguousarray(r.results[0]["res"]).astype(np.float32), r


# revision 22
# speedup vs baseline: 1.4447x; 1.3299x over previous
"""Trainium2 Bass kernel for nn_DiffusionActionHead (B=8, S=2048, D=4096).

Strategy (8 NeuronCores):
  - Data-parallel over batch for everything touching llm_output (32 MiB/core).
  - Tensor-parallel weight reads: core i reads column-slice i of wq/wk/wv,
    row-slice i of wo, column/row slice i of mlp_w1/mlp_w2 (~96 MiB of
    weights split 8 ways), tiny diffusion tail replicated.
  - MAP-head attention with q_len=1 is collapsed algebraically:
        scores[s,h] = llm[s,:] . U[:,h],   U[:,h] = wk[:,hb] @ q_h / sqrt(DH)
        pooled[h,:] = softmax(scores)[h,:] @ llm
        ctx[hb]     = pooled[h,:] @ wv[:,hb] + bv[hb]
    (bk shifts scores by a per-head constant -> cancels in softmax.)
  - 4 small collectives: AllGather(U cols), AllToAll(pooled, head<->batch),
    AllReduce(attn_out partial), AllReduce(mlp partial).
Activations are kept feature-on-partition ("transposed") so every big matmul
streams the weight slice in its natural DRAM layout as the moving operand.
"""

import numpy as np
import ml_dtypes
import sys

if "/opt/trn_rl_repo" not in sys.path:
    sys.path.insert(0, "/opt/trn_rl_repo")

import concourse.bass as bass
import concourse.tile as tile
from concourse import bacc, mybir
from concourse.masks import make_identity
from concourse.bass_utils import run_bass_kernel_spmd

F32 = mybir.dt.float32
F32R = mybir.dt.float32r
BF16 = mybir.dt.bfloat16
F16 = mybir.dt.float16
AF = mybir.ActivationFunctionType
ALU = mybir.AluOpType

B, S, D = 8, 2048, 4096
H, AD, TD, HID, NBLK = 8, 7, 32, 256, 3
DH = D // H            # 512
NC = 8                 # cores
P = 128
SC = S // P            # 16 S-chunks
DC = D // P            # 32 D-chunks
HD2 = D // 2           # 2048 (half width for 4-bank PSUM tiles)
F1S = 4 * D // NC      # 2048 per-core hidden cols of mlp_w1
RSQRT_DH = 1.0 / float(np.sqrt(DH))
TWO_PI = 2.0 * float(np.pi)

# matmul dtype knob per family: "f32" (exact, 4 cyc/row) or "f32r" (fast).
MM_KNOB = {
    "q": "f32r", "u": "f32r", "pooled": "f32r", "ctx": "f32r",
    "attn": "f32r", "mm1": "f32r", "mm2": "f32r", "rin": "f32r",
    "tail": "f32",
}


def _mm(ap, fam):
    if MM_KNOB[fam] == "f32r" and ap.dtype == F32:
        return ap.bitcast(F32R)
    return ap


def _bcast(src_ap, nparts):
    """Partition-broadcast a (1, N) DRAM AP to (nparts, N)."""
    ap = src_ap
    assert ap.shape[0] == 1, ap.shape
    return bass.AP(tensor=ap.tensor, offset=ap.offset,
                   ap=[[0, nparts]] + [list(x) for x in ap.ap[1:]])


def build_program():
    nc = bacc.Bacc("TRN2", target_bir_lowering=False, debug=False,
                   num_devices=NC)

    t = {}

    def din(name, shape, dtype=F32):
        t[name] = nc.dram_tensor(name, shape, dtype, kind="ExternalInput")

    din("llm", [S, D], F16); din("llmT", [D, S], F16)
    din("wq_s", [D, DH], F16); din("bq_s", [1, DH])
    din("wkT_s", [DH, D], F16)
    din("wv_s", [D, DH], F16); din("bv_s", [1, DH])
    din("wo_s", [DH, D], F16); din("bo_r", [1, D])
    din("ln_g_r", [P, DC]); din("ln_b_r", [P, DC])
    din("w1_s", [D, F1S], F16); din("b1_s", [1, F1S])
    din("w2_s", [F1S, D], F16); din("b2_r", [1, D])
    din("probe_r", [P, DC], F16)
    din("four_w2", [TD, 1]); din("phase2", [TD, 1])
    din("timeT", [1, B]); din("naT", [AD, B])
    din("cond_w1", [TD, 2 * TD]); din("cond_b1c", [2 * TD, 1])
    din("cond_w2", [2 * TD, TD]); din("cond_b2c", [TD, 1])
    din("rin_cond", [TD, HID]); din("rin_pool", [D, HID])
    din("rin_na", [AD, HID]); din("rin_b", [1, HID])
    din("blk_ln_g", [NBLK, HID]); din("blk_ln_b", [NBLK, HID])
    din("blk_w1", [NBLK, HID, 4 * HID]); din("blk_b1", [NBLK, 4 * HID])
    din("blk_w2", [NBLK, 4 * HID, HID]); din("blk_b2", [NBLK, HID])
    din("out_w", [HID, AD]); din("out_bc", [1, AD])
    t["res"] = nc.dram_tensor("res", [B, AD], F32, kind="ExternalOutput")

    # collective bounce buffers (internal DRAM; outputs in Shared space)
    t["cc_u_in"] = nc.dram_tensor("cc_u_in", [1, D], F32)
    t["cc_u_out"] = nc.dram_tensor("cc_u_out", [NC, D], F32, addr_space="Shared")
    t["cc_pool_in"] = nc.dram_tensor("cc_pool_in", [H, D], F32)
    t["cc_pool_out"] = nc.dram_tensor("cc_pool_out", [B, D], F32)
    t["cc_attn_in"] = nc.dram_tensor("cc_attn_in", [B, D], F32)
    t["cc_attn_out"] = nc.dram_tensor("cc_attn_out", [B, D], F32,
                                      addr_space="Shared")
    t["cc_mlp_in"] = nc.dram_tensor("cc_mlp_in", [B, D], F32)
    t["cc_mlp_out"] = nc.dram_tensor("cc_mlp_out", [B, D], F32,
                                     addr_space="Shared")

    with tile.TileContext(nc) as tc:
        import contextlib
        with contextlib.ExitStack() as ctx:
            _build(nc, tc, t, ctx)
    nc.finalize()
    return nc


def _build(nc, tc, t, ctx):
    GROUPS = [list(range(NC))]

    singles = ctx.enter_context(tc.tile_pool(name="singles", bufs=1))
    llm_pool = ctx.enter_context(tc.tile_pool(name="llm_pool", bufs=4))
    llmT_pool = ctx.enter_context(tc.tile_pool(name="llmT_pool", bufs=4))
    wst = ctx.enter_context(tc.tile_pool(name="wst", bufs=6))
    nat16 = ctx.enter_context(tc.tile_pool(name="nat16", bufs=2))
    nat8 = ctx.enter_context(tc.tile_pool(name="nat8", bufs=2))
    bcp = ctx.enter_context(tc.tile_pool(name="bcp", bufs=1))
    psA = ctx.enter_context(tc.tile_pool(name="psA", bufs=1, space="PSUM"))
    psB = ctx.enter_context(tc.tile_pool(name="psB", bufs=2, space="PSUM"))
    psC = ctx.enter_context(tc.tile_pool(name="psC", bufs=2, space="PSUM"))

    ident = singles.tile([P, P], F32)
    make_identity(nc, ident)
    eps_sb = singles.tile([P, 1], F32)
    nc.vector.memset(eps_sb[:], 1e-5)

    def evict(dst, src):
        nc.vector.tensor_copy(out=dst, in_=src)

    def t_nat_to_T(src_nat, dst_T, nchunks, npart, uid):
        """(npart, nchunks*128) sbuf -> (128, nchunks, npart) sbuf via PE."""
        for c in range(nchunks):
            ps = psB.tile([P, 8], F32, tag="tp8", name=f"tp_{uid}_{c}")
            nc.tensor.transpose(ps[:, :npart], src_nat[:, c * P:(c + 1) * P],
                                ident[:npart, :npart])
            evict(dst_T[:, c, :], ps[:, :npart])

    def layernorm_nat(x_nat, npart, n, y_nat, uid, eps=1e-5):
        """y = (x - mean) / sqrt(var + eps) over free dim of (npart, n)."""
        nsub = max(1, n // 512)
        st = nat8.tile([npart, nsub, nc.vector.BN_STATS_DIM], F32, tag="lnst",
                       name=f"lnst_{uid}")
        xg = x_nat.rearrange("p (a b) -> p a b", a=nsub)
        for g in range(nsub):
            nc.vector.bn_stats(out=st[:, g, :], in_=xg[:, g, :])
        mv = nat8.tile([npart, nc.vector.BN_AGGR_DIM], F32, tag="lnmv",
                       name=f"lnmv_{uid}")
        nc.vector.bn_aggr(out=mv[:], in_=st[:])
        std = nat8.tile([npart, 1], F32, tag="lnsd", name=f"lnsd_{uid}")
        nc.scalar.activation(out=std[:], in_=mv[:, 1:2], func=AF.Sqrt,
                             bias=eps_sb[:npart, :])
        nc.vector.reciprocal(out=std[:], in_=std[:])
        nc.vector.tensor_scalar(out=y_nat, in0=x_nat, scalar1=mv[:, 0:1],
                                scalar2=std[:], op0=ALU.subtract, op1=ALU.mult)

    # =======================================================================
    # STEP 1: q_s = (probe @ wq_s + bq_s) / sqrt(DH)   -> (1, 512) natural
    # =======================================================================
    probe_sb = singles.tile([P, DC], F16)
    nc.sync.dma_start(out=probe_sb[:], in_=t["probe_r"][:])
    bq_sb = singles.tile([1, DH], F32)
    nc.sync.dma_start(out=bq_sb[:], in_=t["bq_s"][:])

    q_nat = singles.tile([1, DH], F32)
    ps_q = psC.tile([1, DH], F32, tag="vec", name="ps_q")
    for k in range(DC):
        wt = wst.tile([P, HD2], F16, tag="wst", name=f"wq_t{k}")
        nc.sync.dma_start(out=wt[:, :DH], in_=t["wq_s"][k * P:(k + 1) * P, :])
        nc.tensor.matmul(ps_q[:], _mm(probe_sb[:, k:k + 1], "q"),
                         _mm(wt[:, :DH], "q"),
                         start=(k == 0), stop=(k == DC - 1))
    nc.vector.tensor_add(out=q_nat[:], in0=ps_q[:], in1=bq_sb[:])
    nc.vector.tensor_scalar_mul(out=q_nat[:], in0=q_nat[:], scalar1=RSQRT_DH)

    qT = singles.tile([P, DH // P], F16)  # (128, 4)
    for c in range(DH // P):
        ps = psB.tile([P, 8], F32, tag="tp8", name=f"tp_q_{c}")
        nc.tensor.transpose(ps[:, :1], q_nat[:, c * P:(c + 1) * P], ident[:1, :1])
        evict(qT[:, c:c + 1], ps[:, :1])

    # =======================================================================
    # STEP 2: U column of this core's head: U = wkT_s.T @ q~  -> (1, 4096)
    #         AllGather -> cc_u_out (8, 4096) = U.T with one row per head
    # =======================================================================
    u_nat = singles.tile([1, D], F32)
    for nhalf in range(2):
        wk_tiles = []
        for k in range(DH // P):
            wt = wst.tile([P, HD2], F16, tag="wst", name=f"wk_t{nhalf}_{k}")
            nc.sync.dma_start(
                out=wt[:],
                in_=t["wkT_s"][k * P:(k + 1) * P, nhalf * HD2:(nhalf + 1) * HD2])
            wk_tiles.append(wt)
        for ncol in range(4):
            n0 = nhalf * 4 + ncol
            ps_u = psC.tile([1, DH], F32, tag="vec", name=f"ps_u_{n0}")
            for k in range(DH // P):
                nc.tensor.matmul(
                    ps_u[:], _mm(qT[:, k:k + 1], "u"),
                    _mm(wk_tiles[k][:, ncol * DH:(ncol + 1) * DH], "u"),
                    start=(k == 0), stop=(k == DH // P - 1))
            evict(u_nat[:, n0 * DH:(n0 + 1) * DH], ps_u[:])

    nc.gpsimd.dma_start(out=t["cc_u_in"][:], in_=u_nat[:])
    nc.gpsimd.collective_compute(
        "AllGather", ALU.bypass, replica_groups=GROUPS,
        ins=[t["cc_u_in"][:].opt()], outs=[t["cc_u_out"][:].opt()])

    # read back U.T (8, 4096), transpose to (128, 32, 8), cast to bf16
    uh_nat = nat16.tile([H, D], F32, tag="nat16", name="uh_nat")
    nc.sync.dma_start(out=uh_nat[:], in_=t["cc_u_out"][:])
    u_bf = singles.tile([P, DC, H], F16)
    for c in range(DC):
        ps = psB.tile([P, 8], F32, tag="tp8", name=f"tp_u_{c}")
        nc.tensor.transpose(ps[:, :H], uh_nat[:, c * P:(c + 1) * P],
                            ident[:H, :H])
        evict(u_bf[:, c, :], ps[:, :H])

    # =======================================================================
    # STEP 3: scoresT (8, 2048) = U.T @ llmT  (bf16 inputs, fp32 accum)
    # =======================================================================
    ps_sc = psA.tile([H, S], F32, tag="big", name="ps_sc")
    for k in range(DC):
        lt = llmT_pool.tile([P, S], F16, tag="llmT", name=f"llmT_t{k}")
        nc.sync.dma_start(out=lt[:], in_=t["llmT"][k * P:(k + 1) * P, :])
        for n in range(S // 512):
            nc.tensor.matmul(ps_sc[:, n * 512:(n + 1) * 512],
                             u_bf[:, k, :], lt[:, n * 512:(n + 1) * 512],
                             start=(k == 0), stop=(k == DC - 1))

    # =======================================================================
    # STEP 4: softmax over S. Max-subtraction is skipped deliberately:
    # softmax is shift-invariant and |scores| here is < ~1, so exp() is
    # perfectly conditioned; result is mathematically identical.
    # =======================================================================
    p_nat = nat8.tile([H, S], F32, tag="nat8", name="p_nat")
    nc.scalar.activation(out=p_nat[:], in_=ps_sc[:], func=AF.Exp)
    den = singles.tile([H, 1], F32)
    nc.vector.reduce_sum(out=den[:], in_=p_nat[:], axis=mybir.AxisListType.X)
    nc.vector.reciprocal(out=den[:], in_=den[:])
    nc.vector.tensor_scalar_mul(out=p_nat[:], in0=p_nat[:], scalar1=den[:])
    pT = singles.tile([P, SC, H], F16)
    t_nat_to_T(p_nat, pT, SC, H, "p")

    # =======================================================================
    # STEP 5: pooled (8, 4096) = pT.T @ llm ; AllToAll (head <-> batch)
    # =======================================================================
    pooled_nat = nat16.tile([H, D], F32, tag="nat16", name="pooled_nat")
    for half in range(2):
        ps_p = psA.tile([H, HD2], F32, tag="big", name=f"ps_pool_{half}")
        for s in range(SC):
            lt = llm_pool.tile([P, HD2], F16, tag="llm", name=f"llm_t{half}_{s}")
            nc.sync.dma_start(
                out=lt[:],
                in_=t["llm"][s * P:(s + 1) * P, half * HD2:(half + 1) * HD2])
            for n in range(HD2 // 512):
                nc.tensor.matmul(ps_p[:, n * 512:(n + 1) * 512],
                                 _mm(pT[:, s, :], "pooled"),
                                 _mm(lt[:, n * 512:(n + 1) * 512], "pooled"),
                                 start=(s == 0), stop=(s == SC - 1))
        evict(pooled_nat[:, half * HD2:(half + 1) * HD2], ps_p[:])

    nc.gpsimd.dma_start(out=t["cc_pool_in"][:], in_=pooled_nat[:])
    nc.gpsimd.collective_compute(
        "AllToAll", ALU.bypass, replica_groups=GROUPS,
        ins=[t["cc_pool_in"][:].opt()], outs=[t["cc_pool_out"][:].opt()])

    # =======================================================================
    # STEP 6: ctx for this core's head, all batches: (8, 512)
    # =======================================================================
    poolh_nat = nat16.tile([B, D], F32, tag="nat16", name="poolh_nat")
    nc.sync.dma_start(out=poolh_nat[:], in_=t["cc_pool_out"][:])
    poolhT = singles.tile([P, DC, B], F16)
    t_nat_to_T(poolh_nat, poolhT, DC, B, "ph")

    bv_bc = bcp.tile([B, D], F32, tag="bc", name="bv_bc")
    nc.gpsimd.dma_start(out=bv_bc[:, :DH], in_=_bcast(t["bv_s"][:], B))
    ps_cx = psA.tile([B, DH], F32, tag="big", name="ps_cx")
    for k in range(DC):
        wt = wst.tile([P, HD2], F16, tag="wst", name=f"wv_t{k}")
        nc.sync.dma_start(out=wt[:, :DH], in_=t["wv_s"][k * P:(k + 1) * P, :])
        nc.tensor.matmul(ps_cx[:], _mm(poolhT[:, k, :], "ctx"),
                         _mm(wt[:, :DH], "ctx"),
                         start=(k == 0), stop=(k == DC - 1))
    ctx_nat = nat8.tile([B, DH], F32, tag="nat8", name="ctx_nat")
    nc.vector.tensor_add(out=ctx_nat[:], in0=ps_cx[:], in1=bv_bc[:, :DH])
    ctxT = singles.tile([P, DH // P, B], F16)
    t_nat_to_T(ctx_nat, ctxT, DH // P, B, "cx")

    # =======================================================================
    # STEP 7: attn_out partial (8, 4096) = ctx @ wo_s ; AllReduce
    # =======================================================================
    attn_part = nat16.tile([B, D], F32, tag="nat16", name="attn_part")
    for half in range(2):
        ps_a = psA.tile([B, HD2], F32, tag="big", name=f"ps_attn_{half}")
        for k in range(DH // P):
            wt = wst.tile([P, HD2], F16, tag="wst", name=f"wo_t{half}_{k}")
            nc.sync.dma_start(
                out=wt[:],
                in_=t["wo_s"][k * P:(k + 1) * P, half * HD2:(half + 1) * HD2])
            for n in range(HD2 // 512):
                nc.tensor.matmul(ps_a[:, n * 512:(n + 1) * 512],
                                 _mm(ctxT[:, k, :], "attn"),
                                 _mm(wt[:, n * 512:(n + 1) * 512], "attn"),
                                 start=(k == 0), stop=(k == DH // P - 1))
        evict(attn_part[:, half * HD2:(half + 1) * HD2], ps_a[:])
    nc.gpsimd.dma_start(out=t["cc_attn_in"][:], in_=attn_part[:])
    nc.gpsimd.collective_compute(
        "AllReduce", ALU.add, replica_groups=GROUPS,
        ins=[t["cc_attn_in"][:].opt()], outs=[t["cc_attn_out"][:].opt()])

    # =======================================================================
    # STEP 8: attn_out = AR + bo ; y = LN(attn_out)*g+b ; mlp partial ; AR
    # =======================================================================
    attn_nat = singles.tile([B, D], F32)  # persists (residual)
    nc.sync.dma_start(out=attn_nat[:], in_=t["cc_attn_out"][:])
    bo_bc = bcp.tile([B, D], F32, tag="bc", name="bo_bc")
    nc.gpsimd.dma_start(out=bo_bc[:], in_=_bcast(t["bo_r"][:], B))
    nc.vector.tensor_add(out=attn_nat[:], in0=attn_nat[:], in1=bo_bc[:])

    y_nat = nat16.tile([B, D], F32, tag="nat16", name="y_nat")
    layernorm_nat(attn_nat[:], B, D, y_nat[:], "ln0")
    yT = singles.tile([P, DC, B], F16)
    t_nat_to_T(y_nat, yT, DC, B, "y")
    # LN affine in T layout (gamma/beta become per-partition scalars)
    lng_sb = singles.tile([P, DC], F32)
    nc.sync.dma_start(out=lng_sb[:], in_=t["ln_g_r"][:])
    lnb_sb = singles.tile([P, DC], F32)
    nc.sync.dma_start(out=lnb_sb[:], in_=t["ln_b_r"][:])
    for c in range(DC):
        nc.vector.tensor_scalar(out=yT[:, c, :], in0=yT[:, c, :],
                                scalar1=lng_sb[:, c:c + 1],
                                scalar2=lnb_sb[:, c:c + 1],
                                op0=ALU.mult, op1=ALU.add)

    # mm1: h1 (8, 2048) = y @ w1_s ; + b1 ; exact gelu
    ps_h1 = psA.tile([B, F1S], F32, tag="big", name="ps_h1")
    for k in range(DC):
        wt = wst.tile([P, F1S], F16, tag="wst", name=f"w1_t{k}")
        nc.sync.dma_start(out=wt[:], in_=t["w1_s"][k * P:(k + 1) * P, :])
        for n in range(F1S // 512):
            nc.tensor.matmul(ps_h1[:, n * 512:(n + 1) * 512],
                             _mm(yT[:, k, :], "mm1"),
                             _mm(wt[:, n * 512:(n + 1) * 512], "mm1"),
                             start=(k == 0), stop=(k == DC - 1))
    b1_bc = bcp.tile([B, F1S], F32, tag="bc2", name="b1_bc")
    nc.gpsimd.dma_start(out=b1_bc[:], in_=_bcast(t["b1_s"][:], B))
    g_nat = nat8.tile([B, F1S], F32, tag="nat8", name="g_nat")
    nc.vector.tensor_add(out=g_nat[:], in0=ps_h1[:], in1=b1_bc[:])
    nc.scalar.activation(out=g_nat[:], in_=g_nat[:], func=AF.Gelu)
    gT = singles.tile([P, F1S // P, B], F16)
    t_nat_to_T(g_nat, gT, F1S // P, B, "g")

    # mm2: h2 partial (8, 4096) = g @ w2_s ; AllReduce
    h2_nat = nat16.tile([B, D], F32, tag="nat16", name="h2_nat")
    for half in range(2):
        ps_h2 = psA.tile([B, HD2], F32, tag="big", name=f"ps_h2_{half}")
        for k in range(F1S // P):
            wt = wst.tile([P, HD2], F16, tag="wst", name=f"w2_t{half}_{k}")
            nc.sync.dma_start(
                out=wt[:],
                in_=t["w2_s"][k * P:(k + 1) * P, half * HD2:(half + 1) * HD2])
            for n in range(HD2 // 512):
                nc.tensor.matmul(ps_h2[:, n * 512:(n + 1) * 512],
                                 _mm(gT[:, k, :], "mm2"),
                                 _mm(wt[:, n * 512:(n + 1) * 512], "mm2"),
                                 start=(k == 0), stop=(k == F1S // P - 1))
        evict(h2_nat[:, half * HD2:(half + 1) * HD2], ps_h2[:])
    nc.gpsimd.dma_start(out=t["cc_mlp_in"][:], in_=h2_nat[:])
    nc.gpsimd.collective_compute(
        "AllReduce", ALU.add, replica_groups=GROUPS,
        ins=[t["cc_mlp_in"][:].opt()], outs=[t["cc_mlp_out"][:].opt()])

    # =======================================================================
    # STEP 9: x_pool = attn_out + h + b2 ; diffusion tail (replicated)
    # =======================================================================
    hug = nat16.tile([B, D], F32, tag="nat16", name="hug")
    nc.sync.dma_start(out=hug[:], in_=t["cc_mlp_out"][:])
    b2_bc = bcp.tile([B, D], F32, tag="bc", name="b2_bc")
    nc.gpsimd.dma_start(out=b2_bc[:], in_=_bcast(t["b2_r"][:], B))
    nc.vector.tensor_add(out=hug[:], in0=hug[:], in1=b2_bc[:])
    nc.vector.tensor_add(out=attn_nat[:], in0=attn_nat[:], in1=hug[:])
    xpT = singles.tile([P, DC, B], F32R)
    t_nat_to_T(attn_nat, xpT, DC, B, "xp")

    # ---- cond (fourier features + tiny mlp), all batches ----
    # rows 0-15: cos = sin(2*pi*v + pi/2); rows 16-31: sin. One activation
    # with a per-partition phase bias (partition-offset writes must be
    # 32-aligned, so the two halves cannot be written separately).
    fw_sb = singles.tile([TD, 1], F32)
    nc.sync.dma_start(out=fw_sb[:], in_=t["four_w2"][:])
    ph_sb = singles.tile([TD, 1], F32)
    nc.sync.dma_start(out=ph_sb[:], in_=t["phase2"][:])
    tb32 = singles.tile([TD, B], F32)
    nc.gpsimd.dma_start(out=tb32[:], in_=_bcast(t["timeT"][:], TD))
    fu = singles.tile([TD, B], F32)
    nc.vector.tensor_scalar_mul(out=fu[:], in0=tb32[:], scalar1=fw_sb[:])
    # exact range reduction: sin/cos have period 1 in fu, so subtract the
    # integer part via an f32->i32->f32 round-trip (|fu| < ~64 here).
    fi = singles.tile([TD, B], mybir.dt.int32)
    nc.vector.tensor_copy(out=fi[:], in_=fu[:])
    fif = singles.tile([TD, B], F32)
    nc.vector.tensor_copy(out=fif[:], in_=fi[:])
    nc.vector.tensor_sub(out=fu[:], in0=fu[:], in1=fif[:])
    ffT = singles.tile([TD, B], F32)
    nc.scalar.activation(out=ffT[:], in_=fu[:], func=AF.Sin,
                         scale=TWO_PI, bias=ph_sb[:])
    cw1_sb = singles.tile([TD, 2 * TD], F32)
    nc.sync.dma_start(out=cw1_sb[:], in_=t["cond_w1"][:])
    cb1_sb = singles.tile([2 * TD, 1], F32)
    nc.sync.dma_start(out=cb1_sb[:], in_=t["cond_b1c"][:])
    cw2_sb = singles.tile([2 * TD, TD], F32)
    nc.sync.dma_start(out=cw2_sb[:], in_=t["cond_w2"][:])
    cb2_sb = singles.tile([TD, 1], F32)
    nc.sync.dma_start(out=cb2_sb[:], in_=t["cond_b2c"][:])

    ps_c1 = psB.tile([P, 8], F32, tag="tp8", name="ps_c1")
    nc.tensor.matmul(ps_c1[:2 * TD, :B], cw1_sb[:], ffT[:], start=True, stop=True)
    c1 = singles.tile([2 * TD, B], F32)
    nc.scalar.activation(out=c1[:], in_=ps_c1[:2 * TD, :B], func=AF.Silu,
                         bias=cb1_sb[:])
    ps_c2 = psB.tile([P, 8], F32, tag="tp8", name="ps_c2")
    nc.tensor.matmul(ps_c2[:TD, :B], cw2_sb[:], c1[:], start=True, stop=True)
    condT = singles.tile([TD, B], F32)
    nc.scalar.activation(out=condT[:], in_=ps_c2[:TD, :B], func=AF.Identity,
                         bias=cb2_sb[:])

    naT_sb = singles.tile([AD, B], F32)
    nc.sync.dma_start(out=naT_sb[:], in_=t["naT"][:])

    # ---- x0 (8, 256) = cond@rin_cond + x_pool@rin_pool + na@rin_na + rin_b
    ps_x0 = psA.tile([B, HID], F32, tag="big", name="ps_x0")
    for k in range(DC):
        wt = wst.tile([P, HID], F32R, tag="wst", name=f"rp_t{k}")
        nc.sync.dma_start(out=wt[:],
                          in_=t["rin_pool"][k * P:(k + 1) * P, :].bitcast(F32R))
        nc.tensor.matmul(ps_x0[:], _mm(xpT[:, k, :], "rin"), _mm(wt[:], "rin"),
                         start=(k == 0), stop=False)
    rc_sb = singles.tile([TD, HID], F32)
    nc.sync.dma_start(out=rc_sb[:], in_=t["rin_cond"][:])
    nc.tensor.matmul(ps_x0[:], condT[:], rc_sb[:], start=False, stop=False)
    rna_sb = singles.tile([AD, HID], F32)
    nc.sync.dma_start(out=rna_sb[:], in_=t["rin_na"][:])
    nc.tensor.matmul(ps_x0[:], naT_sb[:], rna_sb[:], start=False, stop=True)
    rb_bc = bcp.tile([B, HID], F32, tag="bcs", name="rb_bc")
    nc.gpsimd.dma_start(out=rb_bc[:], in_=_bcast(t["rin_b"][:], B))
    x_nat = singles.tile([B, HID], F32)
    nc.vector.tensor_add(out=x_nat[:], in0=ps_x0[:], in1=rb_bc[:])

    # ---- 3 residual blocks ----
    for i in range(NBLK):
        xn = singles.tile([B, HID], F32, name=f"xn_{i}")
        layernorm_nat(x_nat[:], B, HID, xn[:], f"lnb{i}")
        g_bc = bcp.tile([B, HID], F32, tag="bcs", name=f"bg_bc{i}")
        nc.gpsimd.dma_start(out=g_bc[:], in_=_bcast(t["blk_ln_g"][i:i + 1, :], B))
        b_bc = bcp.tile([B, HID], F32, tag="bcs", name=f"bb_bc{i}")
        nc.gpsimd.dma_start(out=b_bc[:], in_=_bcast(t["blk_ln_b"][i:i + 1, :], B))
        nc.vector.tensor_mul(out=xn[:], in0=xn[:], in1=g_bc[:])
        nc.vector.tensor_add(out=xn[:], in0=xn[:], in1=b_bc[:])
        xnT = singles.tile([P, HID // P, B], F32, name=f"xnT_{i}")
        t_nat_to_T(xn, xnT, HID // P, B, f"xn{i}")

        ps_bh = psA.tile([B, 4 * HID], F32, tag="big", name=f"ps_bh_{i}")
        for k in range(HID // P):
            wt = wst.tile([P, 4 * HID], F32, tag="wst", name=f"bw1_t{i}_{k}")
            nc.sync.dma_start(out=wt[:], in_=t["blk_w1"][i, k * P:(k + 1) * P, :])
            for n in range(4 * HID // 512):
                nc.tensor.matmul(ps_bh[:, n * 512:(n + 1) * 512],
                                 _mm(xnT[:, k, :], "tail"),
                                 _mm(wt[:, n * 512:(n + 1) * 512], "tail"),
                                 start=(k == 0), stop=(k == HID // P - 1))
        hb_bc = bcp.tile([B, 4 * HID], F32, tag="bcs", name=f"b1_bc{i}")
        nc.gpsimd.dma_start(out=hb_bc[:], in_=_bcast(t["blk_b1"][i:i + 1, :], B))
        hb = nat8.tile([B, 4 * HID], F32, tag="nat8", name=f"hb_{i}")
        nc.vector.tensor_add(out=hb[:], in0=ps_bh[:], in1=hb_bc[:])
        nc.scalar.activation(out=hb[:], in_=hb[:], func=AF.Silu)
        hbT = singles.tile([P, 4 * HID // P, B], F32, name=f"hbT_{i}")
        t_nat_to_T(hb, hbT, 4 * HID // P, B, f"hb{i}")

        ps_bo = psA.tile([B, HID], F32, tag="big", name=f"ps_bo_{i}")
        for k in range(4 * HID // P):
            wt = wst.tile([P, HID], F32, tag="wst", name=f"bw2_t{i}_{k}")
            nc.sync.dma_start(out=wt[:], in_=t["blk_w2"][i, k * P:(k + 1) * P, :])
            nc.tensor.matmul(ps_bo[:], _mm(hbT[:, k, :], "tail"),
                             _mm(wt[:], "tail"),
                             start=(k == 0), stop=(k == 4 * HID // P - 1))
        b2b = bcp.tile([B, HID], F32, tag="bcs", name=f"b2_bc{i}")
        nc.gpsimd.dma_start(out=b2b[:], in_=_bcast(t["blk_b2"][i:i + 1, :], B))
        nc.vector.tensor_add(out=b2b[:], in0=ps_bo[:], in1=b2b[:])
        nc.vector.tensor_add(out=x_nat[:], in0=x_nat[:], in1=b2b[:])

    # ---- final: res (7, 8) = (swish(x) @ out_w + out_b).T
    nc.scalar.activation(out=x_nat[:], in_=x_nat[:], func=AF.Silu)
    xsT = singles.tile([P, HID // P, B], F32)
    t_nat_to_T(x_nat, xsT, HID // P, B, "xs")
    ow_sb = singles.tile([P, HID // P, AD], F32)
    nc.sync.dma_start(out=ow_sb[:],
                      in_=t["out_w"][:].rearrange("(c p) a -> p c a", p=P))
    ob_bc = singles.tile([B, AD], F32)
    nc.gpsimd.dma_start(out=ob_bc[:], in_=_bcast(t["out_bc"][:], B))
    ps_o = psB.tile([P, 8], F32, tag="tp8", name="ps_o")
    for k in range(HID // P):
        nc.tensor.matmul(ps_o[:B, :AD], _mm(xsT[:, k, :], "tail"),
                         _mm(ow_sb[:, k, :], "tail"),
                         start=(k == 0), stop=(k == HID // P - 1))
    out_sb = singles.tile([B, AD], F32)
    nc.vector.tensor_add(out=out_sb[:], in0=ps_o[:B, :AD], in1=ob_bc[:])
    nc.sync.dma_start(out=t["res"][:], in_=out_sb[:])


_CACHED_NC = None


def _get_nc():
    global _CACHED_NC
    if _CACHED_NC is None:
        _CACHED_NC = build_program()
    return _CACHED_NC


def _prep_in_maps(inputs):
    f32 = np.float32
    llm_full = np.ascontiguousarray(np.asarray(inputs["llm_output"], dtype=f32))
    wq = np.asarray(inputs["wq"], f32); wk = np.asarray(inputs["wk"], f32)
    wv = np.asarray(inputs["wv"], f32); wo = np.asarray(inputs["wo"], f32)
    bq = np.asarray(inputs["bq"], f32); bv = np.asarray(inputs["bv"], f32)
    bo = np.asarray(inputs["bo"], f32)
    w1 = np.asarray(inputs["mlp_w1"], f32); b1 = np.asarray(inputs["mlp_b1"], f32)
    w2 = np.asarray(inputs["mlp_w2"], f32); b2 = np.asarray(inputs["mlp_b2"], f32)
    rin_w = np.asarray(inputs["rin_w"], f32)
    probe = np.asarray(inputs["probe"], f32).reshape(D)

    def r128(v):  # (n*128,) -> (128, n) partition-major
        return np.ascontiguousarray(v.reshape(-1, P).T)

    shared = {
        "bo_r": bo.reshape(1, D),
        "ln_g_r": r128(np.asarray(inputs["ln_g"], f32)),
        "ln_b_r": r128(np.asarray(inputs["ln_b"], f32)),
        "b2_r": b2.reshape(1, D),
        "probe_r": r128(probe).astype(np.float16),
        "four_w2": np.concatenate(
            [np.asarray(inputs["four_w"], f32).reshape(TD // 2, 1)] * 2),
        "phase2": np.concatenate(
            [np.full((TD // 2, 1), np.pi / 2, f32),
             np.zeros((TD // 2, 1), f32)]),
        "timeT": np.ascontiguousarray(np.asarray(inputs["time"], f32).T),
        "naT": np.ascontiguousarray(np.asarray(inputs["noisy_actions"], f32).T),
        "cond_w1": np.asarray(inputs["cond_w1"], f32),
        "cond_b1c": np.asarray(inputs["cond_b1"], f32).reshape(-1, 1),
        "cond_w2": np.asarray(inputs["cond_w2"], f32),
        "cond_b2c": np.asarray(inputs["cond_b2"], f32).reshape(-1, 1),
        "rin_cond": np.ascontiguousarray(rin_w[0:TD]),
        "rin_pool": np.ascontiguousarray(rin_w[TD:TD + D]),
        "rin_na": np.ascontiguousarray(rin_w[TD + D:]),
        "rin_b": np.asarray(inputs["rin_b"], f32).reshape(1, HID),
        "blk_ln_g": np.asarray(inputs["blk_ln_g"], f32),
        "blk_ln_b": np.asarray(inputs["blk_ln_b"], f32),
        "blk_w1": np.asarray(inputs["blk_w1"], f32),
        "blk_b1": np.asarray(inputs["blk_b1"], f32),
        "blk_w2": np.asarray(inputs["blk_w2"], f32),
        "blk_b2": np.asarray(inputs["blk_b2"], f32),
        "out_w": np.asarray(inputs["out_w"], f32),
        "out_bc": np.asarray(inputs["out_b"], f32).reshape(1, AD),
    }

    in_maps = []
    for i in range(NC):
        hb = slice(i * DH, (i + 1) * DH)
        fb = slice(i * F1S, (i + 1) * F1S)
        m = dict(shared)
        m["llm"] = llm_full[i].astype(np.float16)
        m["llmT"] = np.ascontiguousarray(llm_full[i].T).astype(np.float16)
        m["wq_s"] = np.ascontiguousarray(wq[:, hb]).astype(np.float16)
        m["bq_s"] = np.ascontiguousarray(bq[hb]).reshape(1, DH)
        m["wkT_s"] = np.ascontiguousarray(wk[:, hb].T).astype(np.float16)
        m["wv_s"] = np.ascontiguousarray(wv[:, hb]).astype(np.float16)
        m["bv_s"] = np.ascontiguousarray(bv[hb]).reshape(1, DH)
        m["wo_s"] = np.ascontiguousarray(wo[hb, :]).astype(np.float16)
        m["w1_s"] = np.ascontiguousarray(w1[:, fb]).astype(np.float16)
        m["b1_s"] = np.ascontiguousarray(b1[fb]).reshape(1, F1S)
        m["w2_s"] = np.ascontiguousarray(w2[fb, :]).astype(np.float16)
        in_maps.append(m)
    return in_maps


def kernel(**inputs):
    nc = _get_nc()
    in_maps = _prep_in_maps(inputs)
    r = run_bass_kernel_spmd(nc, in_maps, core_ids=list(range(NC)))
    return np.ascontiguousarray(r.results[0]["res"]).astype(np.float32)


def run_traced(**inputs):
    """Like kernel() but with NTFF tracing; returns (output, results)."""
    nc = _get_nc()
    in_maps = _prep_in_maps(inputs)
    r = run_bass_kernel_spmd(nc, in_maps, core_ids=list(range(NC)), trace=True)
    return np.ascontiguousarray(r.results[0]["res"]).astype(np.float32), r


# revision 23
# speedup vs baseline: 1.5311x; 1.0598x over previous
"""Trainium2 Bass kernel for nn_DiffusionActionHead (B=8, S=2048, D=4096).

Strategy (8 NeuronCores):
  - Data-parallel over batch for everything touching llm_output (32 MiB/core).
  - Tensor-parallel weight reads: core i reads column-slice i of wq/wk/wv,
    row-slice i of wo, column/row slice i of mlp_w1/mlp_w2 (~96 MiB of
    weights split 8 ways), tiny diffusion tail replicated.
  - MAP-head attention with q_len=1 is collapsed algebraically:
        scores[s,h] = llm[s,:] . U[:,h],   U[:,h] = wk[:,hb] @ q_h / sqrt(DH)
        pooled[h,:] = softmax(scores)[h,:] @ llm
        ctx[hb]     = pooled[h,:] @ wv[:,hb] + bv[hb]
    (bk shifts scores by a per-head constant -> cancels in softmax.)
  - 4 small collectives: AllGather(U cols), AllToAll(pooled, head<->batch),
    AllReduce(attn_out partial), AllReduce(mlp partial).
Activations are kept feature-on-partition ("transposed") so every big matmul
streams the weight slice in its natural DRAM layout as the moving operand.
"""

import numpy as np
import ml_dtypes
import sys

if "/opt/trn_rl_repo" not in sys.path:
    sys.path.insert(0, "/opt/trn_rl_repo")

import concourse.bass as bass
import concourse.tile as tile
from concourse import bacc, mybir
from concourse.masks import make_identity
from concourse.bass_utils import run_bass_kernel_spmd

F32 = mybir.dt.float32
F32R = mybir.dt.float32r
BF16 = mybir.dt.bfloat16
F16 = mybir.dt.float16
AF = mybir.ActivationFunctionType
ALU = mybir.AluOpType

B, S, D = 8, 2048, 4096
H, AD, TD, HID, NBLK = 8, 7, 32, 256, 3
DH = D // H            # 512
NC = 8                 # cores
P = 128
SC = S // P            # 16 S-chunks
DC = D // P            # 32 D-chunks
HD2 = D // 2           # 2048 (half width for 4-bank PSUM tiles)
F1S = 4 * D // NC      # 2048 per-core hidden cols of mlp_w1
RSQRT_DH = 1.0 / float(np.sqrt(DH))
TWO_PI = 2.0 * float(np.pi)

# matmul dtype knob per family: "f32" (exact, 4 cyc/row) or "f32r" (fast).
MM_KNOB = {
    "q": "f32r", "u": "f32r", "pooled": "f32r", "ctx": "f32r",
    "attn": "f32r", "mm1": "f32r", "mm2": "f32r", "rin": "f32r",
    "tail": "f32",
}


def _mm(ap, fam):
    if MM_KNOB[fam] == "f32r" and ap.dtype == F32:
        return ap.bitcast(F32R)
    return ap


def _bcast(src_ap, nparts):
    """Partition-broadcast a (1, N) DRAM AP to (nparts, N)."""
    ap = src_ap
    assert ap.shape[0] == 1, ap.shape
    return bass.AP(tensor=ap.tensor, offset=ap.offset,
                   ap=[[0, nparts]] + [list(x) for x in ap.ap[1:]])


def build_program():
    nc = bacc.Bacc("TRN2", target_bir_lowering=False, debug=False,
                   num_devices=NC)

    t = {}

    def din(name, shape, dtype=F32):
        t[name] = nc.dram_tensor(name, shape, dtype, kind="ExternalInput")

    din("llm", [S, D], F16); din("llmT", [D, S], F16)
    din("wq_s", [D, DH], F16); din("bq_s", [1, DH])
    din("wkT_s", [DH, D], F16)
    din("wv_s", [D, DH], F16); din("bv_s", [1, DH])
    din("wo_s", [DH, D], F16); din("bo_r", [1, D])
    din("ln_g_r", [P, DC]); din("ln_b_r", [P, DC])
    din("w1_s", [D, F1S], F16); din("b1_s", [1, F1S])
    din("w2_s", [F1S, D], F16); din("b2_r", [1, D])
    din("probe_r", [P, DC], F16)
    din("four_w2", [TD, 1]); din("phase2", [TD, 1])
    din("timeT", [1, B]); din("naT", [AD, B], F16)
    din("cond_w1", [TD, 2 * TD], F16); din("cond_b1c", [2 * TD, 1])
    din("cond_w2", [2 * TD, TD], F16); din("cond_b2c", [TD, 1])
    din("rin_cond", [TD, HID], F16); din("rin_pool", [D, HID], F16)
    din("rin_na", [AD, HID], F16); din("rin_b", [1, HID])
    din("blk_ln_g", [NBLK, HID]); din("blk_ln_b", [NBLK, HID])
    din("blk_w1", [NBLK, HID, 4 * HID], F16); din("blk_b1", [NBLK, 4 * HID])
    din("blk_w2", [NBLK, 4 * HID, HID], F16); din("blk_b2", [NBLK, HID])
    din("out_w", [HID, AD], F16); din("out_bc", [1, AD])
    t["res"] = nc.dram_tensor("res", [B, AD], F32, kind="ExternalOutput")

    # collective bounce buffers (internal DRAM; outputs in Shared space)
    t["cc_u_in"] = nc.dram_tensor("cc_u_in", [1, D], F32)
    t["cc_u_out"] = nc.dram_tensor("cc_u_out", [NC, D], F32, addr_space="Shared")
    t["cc_pool_in"] = nc.dram_tensor("cc_pool_in", [H, D], F32)
    t["cc_pool_out"] = nc.dram_tensor("cc_pool_out", [B, D], F32)
    t["cc_attn_in"] = nc.dram_tensor("cc_attn_in", [B, D], F32)
    t["cc_attn_out"] = nc.dram_tensor("cc_attn_out", [B, D], F32,
                                      addr_space="Shared")
    t["cc_mlp_in"] = nc.dram_tensor("cc_mlp_in", [B, D], F32)
    t["cc_mlp_out"] = nc.dram_tensor("cc_mlp_out", [B, D], F32,
                                     addr_space="Shared")

    with tile.TileContext(nc) as tc:
        import contextlib
        with contextlib.ExitStack() as ctx:
            _build(nc, tc, t, ctx)
    nc.finalize()
    return nc


def _build(nc, tc, t, ctx):
    GROUPS = [list(range(NC))]

    singles = ctx.enter_context(tc.tile_pool(name="singles", bufs=1))
    llm_pool = ctx.enter_context(tc.tile_pool(name="llm_pool", bufs=6))
    llmT_pool = ctx.enter_context(tc.tile_pool(name="llmT_pool", bufs=8))
    wst = ctx.enter_context(tc.tile_pool(name="wst", bufs=8))
    nat16 = ctx.enter_context(tc.tile_pool(name="nat16", bufs=2))
    nat8 = ctx.enter_context(tc.tile_pool(name="nat8", bufs=2))
    bcp = ctx.enter_context(tc.tile_pool(name="bcp", bufs=1))
    psA = ctx.enter_context(tc.tile_pool(name="psA", bufs=1, space="PSUM"))
    psB = ctx.enter_context(tc.tile_pool(name="psB", bufs=2, space="PSUM"))
    psC = ctx.enter_context(tc.tile_pool(name="psC", bufs=2, space="PSUM"))

    ident = singles.tile([P, P], F32)
    make_identity(nc, ident)
    eps_sb = singles.tile([P, 1], F32)
    nc.vector.memset(eps_sb[:], 1e-5)

    def evict(dst, src):
        nc.vector.tensor_copy(out=dst, in_=src)

    def t_nat_to_T(src_nat, dst_T, nchunks, npart, uid):
        """(npart, nchunks*128) sbuf -> (128, nchunks, npart) sbuf via PE."""
        for c in range(nchunks):
            ps = psB.tile([P, 8], F32, tag="tp8", name=f"tp_{uid}_{c}")
            nc.tensor.transpose(ps[:, :npart], src_nat[:, c * P:(c + 1) * P],
                                ident[:npart, :npart])
            evict(dst_T[:, c, :], ps[:, :npart])

    def layernorm_nat(x_nat, npart, n, y_nat, uid, eps=1e-5):
        """y = (x - mean) / sqrt(var + eps) over free dim of (npart, n)."""
        nsub = max(1, n // 512)
        st = nat8.tile([npart, nsub, nc.vector.BN_STATS_DIM], F32, tag="lnst",
                       name=f"lnst_{uid}")
        xg = x_nat.rearrange("p (a b) -> p a b", a=nsub)
        for g in range(nsub):
            nc.vector.bn_stats(out=st[:, g, :], in_=xg[:, g, :])
        mv = nat8.tile([npart, nc.vector.BN_AGGR_DIM], F32, tag="lnmv",
                       name=f"lnmv_{uid}")
        nc.vector.bn_aggr(out=mv[:], in_=st[:])
        std = nat8.tile([npart, 1], F32, tag="lnsd", name=f"lnsd_{uid}")
        nc.scalar.activation(out=std[:], in_=mv[:, 1:2], func=AF.Sqrt,
                             bias=eps_sb[:npart, :])
        nc.vector.reciprocal(out=std[:], in_=std[:])
        nc.vector.tensor_scalar(out=y_nat, in0=x_nat, scalar1=mv[:, 0:1],
                                scalar2=std[:], op0=ALU.subtract, op1=ALU.mult)

    # =======================================================================
    # STEP 1: q_s = (probe @ wq_s + bq_s) / sqrt(DH)   -> (1, 512) natural
    # =======================================================================
    probe_sb = singles.tile([P, DC], F16)
    nc.sync.dma_start(out=probe_sb[:], in_=t["probe_r"][:])
    bq_sb = singles.tile([1, DH], F32)
    nc.sync.dma_start(out=bq_sb[:], in_=t["bq_s"][:])

    q_nat = singles.tile([1, DH], F32)
    ps_q = psC.tile([1, DH], F32, tag="vec", name="ps_q")
    for k in range(DC):
        wt = wst.tile([P, HD2], F16, tag="wst", name=f"wq_t{k}")
        nc.scalar.dma_start(out=wt[:, :DH], in_=t["wq_s"][k * P:(k + 1) * P, :])
        nc.tensor.matmul(ps_q[:], _mm(probe_sb[:, k:k + 1], "q"),
                         _mm(wt[:, :DH], "q"),
                         start=(k == 0), stop=(k == DC - 1))
    nc.vector.tensor_add(out=q_nat[:], in0=ps_q[:], in1=bq_sb[:])
    nc.vector.tensor_scalar_mul(out=q_nat[:], in0=q_nat[:], scalar1=RSQRT_DH)

    qT = singles.tile([P, DH // P], F16)  # (128, 4)
    for c in range(DH // P):
        ps = psB.tile([P, 8], F32, tag="tp8", name=f"tp_q_{c}")
        nc.tensor.transpose(ps[:, :1], q_nat[:, c * P:(c + 1) * P], ident[:1, :1])
        evict(qT[:, c:c + 1], ps[:, :1])

    # =======================================================================
    # STEP 2: U column of this core's head: U = wkT_s.T @ q~  -> (1, 4096)
    #         AllGather -> cc_u_out (8, 4096) = U.T with one row per head
    # =======================================================================
    u_nat = nat16.tile([1, D], F32, tag="nat16", name="u_nat")
    for nhalf in range(2):
        wk_tiles = []
        for k in range(DH // P):
            wt = wst.tile([P, HD2], F16, tag="wst", name=f"wk_t{nhalf}_{k}")
            nc.scalar.dma_start(
                out=wt[:],
                in_=t["wkT_s"][k * P:(k + 1) * P, nhalf * HD2:(nhalf + 1) * HD2])
            wk_tiles.append(wt)
        for ncol in range(4):
            n0 = nhalf * 4 + ncol
            ps_u = psC.tile([1, DH], F32, tag="vec", name=f"ps_u_{n0}")
            for k in range(DH // P):
                nc.tensor.matmul(
                    ps_u[:], _mm(qT[:, k:k + 1], "u"),
                    _mm(wk_tiles[k][:, ncol * DH:(ncol + 1) * DH], "u"),
                    start=(k == 0), stop=(k == DH // P - 1))
            evict(u_nat[:, n0 * DH:(n0 + 1) * DH], ps_u[:])

    nc.gpsimd.dma_start(out=t["cc_u_in"][:], in_=u_nat[:])
    nc.gpsimd.collective_compute(
        "AllGather", ALU.bypass, replica_groups=GROUPS,
        ins=[t["cc_u_in"][:].opt()], outs=[t["cc_u_out"][:].opt()])

    # read back U.T (8, 4096), transpose to (128, 32, 8), cast to bf16
    uh_nat = nat16.tile([H, D], F32, tag="nat16", name="uh_nat")
    nc.sync.dma_start(out=uh_nat[:], in_=t["cc_u_out"][:])
    u_bf = singles.tile([P, DC, H], F16)
    for c in range(DC):
        ps = psB.tile([P, 8], F32, tag="tp8", name=f"tp_u_{c}")
        nc.tensor.transpose(ps[:, :H], uh_nat[:, c * P:(c + 1) * P],
                            ident[:H, :H])
        evict(u_bf[:, c, :], ps[:, :H])

    # =======================================================================
    # STEP 3: scoresT (8, 2048) = U.T @ llmT  (bf16 inputs, fp32 accum)
    # =======================================================================
    ps_sc = psA.tile([H, S], F32, tag="big", name="ps_sc")
    for k in range(DC):
        lt = llmT_pool.tile([P, S], F16, tag="llmT", name=f"llmT_t{k}")
        nc.sync.dma_start(out=lt[:], in_=t["llmT"][k * P:(k + 1) * P, :])
        for n in range(S // 512):
            nc.tensor.matmul(ps_sc[:, n * 512:(n + 1) * 512],
                             u_bf[:, k, :], lt[:, n * 512:(n + 1) * 512],
                             start=(k == 0), stop=(k == DC - 1))

    # =======================================================================
    # STEP 4: softmax over S. Max-subtraction is skipped deliberately:
    # softmax is shift-invariant and |scores| here is < ~1, so exp() is
    # perfectly conditioned; result is mathematically identical.
    # =======================================================================
    p_nat = nat8.tile([H, S], F32, tag="nat8", name="p_nat")
    nc.scalar.activation(out=p_nat[:], in_=ps_sc[:], func=AF.Exp)
    den = singles.tile([H, 1], F32)
    nc.vector.reduce_sum(out=den[:], in_=p_nat[:], axis=mybir.AxisListType.X)
    nc.vector.reciprocal(out=den[:], in_=den[:])
    nc.vector.tensor_scalar_mul(out=p_nat[:], in0=p_nat[:], scalar1=den[:])
    pT = singles.tile([P, SC, H], F16)
    t_nat_to_T(p_nat, pT, SC, H, "p")

    # =======================================================================
    # STEP 5: pooled (8, 4096) = pT.T @ llm ; AllToAll (head <-> batch)
    # =======================================================================
    pooled_nat = nat16.tile([H, D], F32, tag="nat16", name="pooled_nat")
    for half in range(2):
        ps_p = psA.tile([H, HD2], F32, tag="big", name=f"ps_pool_{half}")
        for s in range(SC):
            lt = llm_pool.tile([P, HD2], F16, tag="llm", name=f"llm_t{half}_{s}")
            nc.sync.dma_start(
                out=lt[:],
                in_=t["llm"][s * P:(s + 1) * P, half * HD2:(half + 1) * HD2])
            for n in range(HD2 // 512):
                nc.tensor.matmul(ps_p[:, n * 512:(n + 1) * 512],
                                 _mm(pT[:, s, :], "pooled"),
                                 _mm(lt[:, n * 512:(n + 1) * 512], "pooled"),
                                 start=(s == 0), stop=(s == SC - 1))
        evict(pooled_nat[:, half * HD2:(half + 1) * HD2], ps_p[:])

    nc.gpsimd.dma_start(out=t["cc_pool_in"][:], in_=pooled_nat[:])
    nc.gpsimd.collective_compute(
        "AllToAll", ALU.bypass, replica_groups=GROUPS,
        ins=[t["cc_pool_in"][:].opt()], outs=[t["cc_pool_out"][:].opt()])

    # =======================================================================
    # STEP 6: ctx for this core's head, all batches: (8, 512)
    # =======================================================================
    poolh_nat = nat16.tile([B, D], F32, tag="nat16", name="poolh_nat")
    nc.sync.dma_start(out=poolh_nat[:], in_=t["cc_pool_out"][:])
    poolhT = singles.tile([P, DC, B], F16)
    t_nat_to_T(poolh_nat, poolhT, DC, B, "ph")

    bv_bc = bcp.tile([B, D], F32, tag="bc", name="bv_bc")
    nc.gpsimd.dma_start(out=bv_bc[:, :DH], in_=_bcast(t["bv_s"][:], B))
    ps_cx = psA.tile([B, DH], F32, tag="big", name="ps_cx")
    for k in range(DC):
        wt = wst.tile([P, HD2], F16, tag="wst", name=f"wv_t{k}")
        nc.scalar.dma_start(out=wt[:, :DH], in_=t["wv_s"][k * P:(k + 1) * P, :])
        nc.tensor.matmul(ps_cx[:], _mm(poolhT[:, k, :], "ctx"),
                         _mm(wt[:, :DH], "ctx"),
                         start=(k == 0), stop=(k == DC - 1))
    ctx_nat = nat8.tile([B, DH], F32, tag="nat8", name="ctx_nat")
    nc.vector.tensor_add(out=ctx_nat[:], in0=ps_cx[:], in1=bv_bc[:, :DH])
    ctxT = singles.tile([P, DH // P, B], F16)
    t_nat_to_T(ctx_nat, ctxT, DH // P, B, "cx")

    # =======================================================================
    # STEP 7: attn_out partial (8, 4096) = ctx @ wo_s ; AllReduce
    # =======================================================================
    attn_part = nat16.tile([B, D], F32, tag="nat16", name="attn_part")
    for half in range(2):
        ps_a = psA.tile([B, HD2], F32, tag="big", name=f"ps_attn_{half}")
        for k in range(DH // P):
            wt = wst.tile([P, HD2], F16, tag="wst", name=f"wo_t{half}_{k}")
            nc.scalar.dma_start(
                out=wt[:],
                in_=t["wo_s"][k * P:(k + 1) * P, half * HD2:(half + 1) * HD2])
            for n in range(HD2 // 512):
                nc.tensor.matmul(ps_a[:, n * 512:(n + 1) * 512],
                                 _mm(ctxT[:, k, :], "attn"),
                                 _mm(wt[:, n * 512:(n + 1) * 512], "attn"),
                                 start=(k == 0), stop=(k == DH // P - 1))
        evict(attn_part[:, half * HD2:(half + 1) * HD2], ps_a[:])
    nc.gpsimd.dma_start(out=t["cc_attn_in"][:], in_=attn_part[:])
    nc.gpsimd.collective_compute(
        "AllReduce", ALU.add, replica_groups=GROUPS,
        ins=[t["cc_attn_in"][:].opt()], outs=[t["cc_attn_out"][:].opt()])

    # =======================================================================
    # STEP 8: attn_out = AR + bo ; y = LN(attn_out)*g+b ; mlp partial ; AR
    # =======================================================================
    attn_nat = singles.tile([B, D], F32)  # persists (residual)
    nc.sync.dma_start(out=attn_nat[:], in_=t["cc_attn_out"][:])
    bo_bc = bcp.tile([B, D], F32, tag="bc", name="bo_bc")
    nc.gpsimd.dma_start(out=bo_bc[:], in_=_bcast(t["bo_r"][:], B))
    nc.vector.tensor_add(out=attn_nat[:], in0=attn_nat[:], in1=bo_bc[:])

    y_nat = nat16.tile([B, D], F32, tag="nat16", name="y_nat")
    layernorm_nat(attn_nat[:], B, D, y_nat[:], "ln0")
    yT = singles.tile([P, DC, B], F16)
    t_nat_to_T(y_nat, yT, DC, B, "y")
    # LN affine in T layout (gamma/beta become per-partition scalars)
    lng_sb = singles.tile([P, DC], F32)
    nc.sync.dma_start(out=lng_sb[:], in_=t["ln_g_r"][:])
    lnb_sb = singles.tile([P, DC], F32)
    nc.sync.dma_start(out=lnb_sb[:], in_=t["ln_b_r"][:])
    for c in range(DC):
        nc.vector.tensor_scalar(out=yT[:, c, :], in0=yT[:, c, :],
                                scalar1=lng_sb[:, c:c + 1],
                                scalar2=lnb_sb[:, c:c + 1],
                                op0=ALU.mult, op1=ALU.add)

    # mm1: h1 (8, 2048) = y @ w1_s ; + b1 ; exact gelu
    ps_h1 = psA.tile([B, F1S], F32, tag="big", name="ps_h1")
    for k in range(DC):
        wt = wst.tile([P, F1S], F16, tag="wst", name=f"w1_t{k}")
        nc.scalar.dma_start(out=wt[:], in_=t["w1_s"][k * P:(k + 1) * P, :])
        for n in range(F1S // 512):
            nc.tensor.matmul(ps_h1[:, n * 512:(n + 1) * 512],
                             _mm(yT[:, k, :], "mm1"),
                             _mm(wt[:, n * 512:(n + 1) * 512], "mm1"),
                             start=(k == 0), stop=(k == DC - 1))
    b1_bc = bcp.tile([B, F1S], F32, tag="bc2", name="b1_bc")
    nc.gpsimd.dma_start(out=b1_bc[:], in_=_bcast(t["b1_s"][:], B))
    g_nat = nat8.tile([B, F1S], F32, tag="nat8", name="g_nat")
    nc.vector.tensor_add(out=g_nat[:], in0=ps_h1[:], in1=b1_bc[:])
    nc.scalar.activation(out=g_nat[:], in_=g_nat[:], func=AF.Gelu)
    gT = singles.tile([P, F1S // P, B], F16)
    t_nat_to_T(g_nat, gT, F1S // P, B, "g")

    # mm2: h2 partial (8, 4096) = g @ w2_s ; AllReduce
    h2_nat = nat16.tile([B, D], F32, tag="nat16", name="h2_nat")
    for half in range(2):
        ps_h2 = psA.tile([B, HD2], F32, tag="big", name=f"ps_h2_{half}")
        for k in range(F1S // P):
            wt = wst.tile([P, HD2], F16, tag="wst", name=f"w2_t{half}_{k}")
            nc.scalar.dma_start(
                out=wt[:],
                in_=t["w2_s"][k * P:(k + 1) * P, half * HD2:(half + 1) * HD2])
            for n in range(HD2 // 512):
                nc.tensor.matmul(ps_h2[:, n * 512:(n + 1) * 512],
                                 _mm(gT[:, k, :], "mm2"),
                                 _mm(wt[:, n * 512:(n + 1) * 512], "mm2"),
                                 start=(k == 0), stop=(k == F1S // P - 1))
        evict(h2_nat[:, half * HD2:(half + 1) * HD2], ps_h2[:])
    nc.gpsimd.dma_start(out=t["cc_mlp_in"][:], in_=h2_nat[:])
    nc.gpsimd.collective_compute(
        "AllReduce", ALU.add, replica_groups=GROUPS,
        ins=[t["cc_mlp_in"][:].opt()], outs=[t["cc_mlp_out"][:].opt()])

    # =======================================================================
    # STEP 9: x_pool = attn_out + h + b2 ; diffusion tail (replicated)
    # =======================================================================
    hug = nat16.tile([B, D], F32, tag="nat16", name="hug")
    nc.sync.dma_start(out=hug[:], in_=t["cc_mlp_out"][:])
    b2_bc = bcp.tile([B, D], F32, tag="bc", name="b2_bc")
    nc.gpsimd.dma_start(out=b2_bc[:], in_=_bcast(t["b2_r"][:], B))
    nc.vector.tensor_add(out=hug[:], in0=hug[:], in1=b2_bc[:])
    nc.vector.tensor_add(out=attn_nat[:], in0=attn_nat[:], in1=hug[:])
    xpT = singles.tile([P, DC, B], F16)
    t_nat_to_T(attn_nat, xpT, DC, B, "xp")

    # ---- cond (fourier features + tiny mlp), all batches ----
    # rows 0-15: cos = sin(2*pi*v + pi/2); rows 16-31: sin. One activation
    # with a per-partition phase bias (partition-offset writes must be
    # 32-aligned, so the two halves cannot be written separately).
    fw_sb = singles.tile([TD, 1], F32)
    nc.sync.dma_start(out=fw_sb[:], in_=t["four_w2"][:])
    ph_sb = singles.tile([TD, 1], F32)
    nc.sync.dma_start(out=ph_sb[:], in_=t["phase2"][:])
    tb32 = singles.tile([TD, B], F32)
    nc.gpsimd.dma_start(out=tb32[:], in_=_bcast(t["timeT"][:], TD))
    fu = singles.tile([TD, B], F32)
    nc.vector.tensor_scalar_mul(out=fu[:], in0=tb32[:], scalar1=fw_sb[:])
    # exact range reduction: sin/cos have period 1 in fu, so subtract the
    # integer part via an f32->i32->f32 round-trip (|fu| < ~64 here).
    fi = singles.tile([TD, B], mybir.dt.int32)
    nc.vector.tensor_copy(out=fi[:], in_=fu[:])
    fif = singles.tile([TD, B], F32)
    nc.vector.tensor_copy(out=fif[:], in_=fi[:])
    nc.vector.tensor_sub(out=fu[:], in0=fu[:], in1=fif[:])
    ffT = singles.tile([TD, B], F16)
    nc.scalar.activation(out=ffT[:], in_=fu[:], func=AF.Sin,
                         scale=TWO_PI, bias=ph_sb[:])
    cw1_sb = singles.tile([TD, 2 * TD], F16)
    nc.sync.dma_start(out=cw1_sb[:], in_=t["cond_w1"][:])
    cb1_sb = singles.tile([2 * TD, 1], F32)
    nc.sync.dma_start(out=cb1_sb[:], in_=t["cond_b1c"][:])
    cw2_sb = singles.tile([2 * TD, TD], F16)
    nc.sync.dma_start(out=cw2_sb[:], in_=t["cond_w2"][:])
    cb2_sb = singles.tile([TD, 1], F32)
    nc.sync.dma_start(out=cb2_sb[:], in_=t["cond_b2c"][:])

    ps_c1 = psB.tile([P, 8], F32, tag="tp8", name="ps_c1")
    nc.tensor.matmul(ps_c1[:2 * TD, :B], cw1_sb[:], ffT[:], start=True, stop=True)
    c1 = singles.tile([2 * TD, B], F16)
    nc.scalar.activation(out=c1[:], in_=ps_c1[:2 * TD, :B], func=AF.Silu,
                         bias=cb1_sb[:])
    ps_c2 = psB.tile([P, 8], F32, tag="tp8", name="ps_c2")
    nc.tensor.matmul(ps_c2[:TD, :B], cw2_sb[:], c1[:], start=True, stop=True)
    condT = singles.tile([TD, B], F16)
    nc.scalar.activation(out=condT[:], in_=ps_c2[:TD, :B], func=AF.Identity,
                         bias=cb2_sb[:])

    naT_sb = singles.tile([AD, B], F16)
    nc.sync.dma_start(out=naT_sb[:], in_=t["naT"][:])

    # ---- x0 (8, 256) = cond@rin_cond + x_pool@rin_pool + na@rin_na + rin_b
    ps_x0 = psA.tile([B, HID], F32, tag="big", name="ps_x0")
    for k in range(DC):
        wt = wst.tile([P, HID], F16, tag="wst", name=f"rp_t{k}")
        nc.scalar.dma_start(out=wt[:], in_=t["rin_pool"][k * P:(k + 1) * P, :])
        nc.tensor.matmul(ps_x0[:], _mm(xpT[:, k, :], "rin"), _mm(wt[:], "rin"),
                         start=(k == 0), stop=False)
    rc_sb = singles.tile([TD, HID], F16)
    nc.scalar.dma_start(out=rc_sb[:], in_=t["rin_cond"][:])
    nc.tensor.matmul(ps_x0[:], condT[:], rc_sb[:], start=False, stop=False)
    rna_sb = singles.tile([AD, HID], F16)
    nc.scalar.dma_start(out=rna_sb[:], in_=t["rin_na"][:])
    nc.tensor.matmul(ps_x0[:], naT_sb[:], rna_sb[:], start=False, stop=True)
    rb_bc = bcp.tile([B, HID], F32, tag="bcs", name="rb_bc")
    nc.gpsimd.dma_start(out=rb_bc[:], in_=_bcast(t["rin_b"][:], B))
    x_nat = singles.tile([B, HID], F32)
    nc.vector.tensor_add(out=x_nat[:], in0=ps_x0[:], in1=rb_bc[:])

    # ---- 3 residual blocks ----
    for i in range(NBLK):
        xn = singles.tile([B, HID], F32, name=f"xn_{i}")
        layernorm_nat(x_nat[:], B, HID, xn[:], f"lnb{i}")
        g_bc = bcp.tile([B, HID], F32, tag="bcs", name=f"bg_bc{i}")
        nc.gpsimd.dma_start(out=g_bc[:], in_=_bcast(t["blk_ln_g"][i:i + 1, :], B))
        b_bc = bcp.tile([B, HID], F32, tag="bcs", name=f"bb_bc{i}")
        nc.gpsimd.dma_start(out=b_bc[:], in_=_bcast(t["blk_ln_b"][i:i + 1, :], B))
        nc.vector.tensor_mul(out=xn[:], in0=xn[:], in1=g_bc[:])
        nc.vector.tensor_add(out=xn[:], in0=xn[:], in1=b_bc[:])
        xnT = singles.tile([P, HID // P, B], F16, name=f"xnT_{i}")
        t_nat_to_T(xn, xnT, HID // P, B, f"xn{i}")

        ps_bh = psA.tile([B, 4 * HID], F32, tag="big", name=f"ps_bh_{i}")
        for k in range(HID // P):
            wt = wst.tile([P, 4 * HID], F16, tag="wst", name=f"bw1_t{i}_{k}")
            nc.scalar.dma_start(out=wt[:], in_=t["blk_w1"][i, k * P:(k + 1) * P, :])
            for n in range(4 * HID // 512):
                nc.tensor.matmul(ps_bh[:, n * 512:(n + 1) * 512],
                                 _mm(xnT[:, k, :], "tail"),
                                 _mm(wt[:, n * 512:(n + 1) * 512], "tail"),
                                 start=(k == 0), stop=(k == HID // P - 1))
        hb_bc = bcp.tile([B, 4 * HID], F32, tag="bcs", name=f"b1_bc{i}")
        nc.gpsimd.dma_start(out=hb_bc[:], in_=_bcast(t["blk_b1"][i:i + 1, :], B))
        hb = nat8.tile([B, 4 * HID], F32, tag="nat8", name=f"hb_{i}")
        nc.vector.tensor_add(out=hb[:], in0=ps_bh[:], in1=hb_bc[:])
        nc.scalar.activation(out=hb[:], in_=hb[:], func=AF.Silu)
        hbT = singles.tile([P, 4 * HID // P, B], F16, name=f"hbT_{i}")
        t_nat_to_T(hb, hbT, 4 * HID // P, B, f"hb{i}")

        ps_bo = psA.tile([B, HID], F32, tag="big", name=f"ps_bo_{i}")
        for k in range(4 * HID // P):
            wt = wst.tile([P, HID], F16, tag="wst", name=f"bw2_t{i}_{k}")
            nc.scalar.dma_start(out=wt[:], in_=t["blk_w2"][i, k * P:(k + 1) * P, :])
            nc.tensor.matmul(ps_bo[:], _mm(hbT[:, k, :], "tail"),
                             _mm(wt[:], "tail"),
                             start=(k == 0), stop=(k == 4 * HID // P - 1))
        b2b = bcp.tile([B, HID], F32, tag="bcs", name=f"b2_bc{i}")
        nc.gpsimd.dma_start(out=b2b[:], in_=_bcast(t["blk_b2"][i:i + 1, :], B))
        nc.vector.tensor_add(out=b2b[:], in0=ps_bo[:], in1=b2b[:])
        nc.vector.tensor_add(out=x_nat[:], in0=x_nat[:], in1=b2b[:])

    # ---- final: res (7, 8) = (swish(x) @ out_w + out_b).T
    nc.scalar.activation(out=x_nat[:], in_=x_nat[:], func=AF.Silu)
    xsT = singles.tile([P, HID // P, B], F16)
    t_nat_to_T(x_nat, xsT, HID // P, B, "xs")
    ow_sb = singles.tile([P, HID // P, AD], F16)
    nc.sync.dma_start(out=ow_sb[:],
                      in_=t["out_w"][:].rearrange("(c p) a -> p c a", p=P))
    ob_bc = singles.tile([B, AD], F32)
    nc.gpsimd.dma_start(out=ob_bc[:], in_=_bcast(t["out_bc"][:], B))
    ps_o = psB.tile([P, 8], F32, tag="tp8", name="ps_o")
    for k in range(HID // P):
        nc.tensor.matmul(ps_o[:B, :AD], _mm(xsT[:, k, :], "tail"),
                         _mm(ow_sb[:, k, :], "tail"),
                         start=(k == 0), stop=(k == HID // P - 1))
    out_sb = singles.tile([B, AD], F32)
    nc.vector.tensor_add(out=out_sb[:], in0=ps_o[:B, :AD], in1=ob_bc[:])
    nc.sync.dma_start(out=t["res"][:], in_=out_sb[:])


_CACHED_NC = None


def _get_nc():
    global _CACHED_NC
    if _CACHED_NC is None:
        _CACHED_NC = build_program()
    return _CACHED_NC


def _prep_in_maps(inputs):
    f32 = np.float32
    llm_full = np.ascontiguousarray(np.asarray(inputs["llm_output"], dtype=f32))
    wq = np.asarray(inputs["wq"], f32); wk = np.asarray(inputs["wk"], f32)
    wv = np.asarray(inputs["wv"], f32); wo = np.asarray(inputs["wo"], f32)
    bq = np.asarray(inputs["bq"], f32); bv = np.asarray(inputs["bv"], f32)
    bo = np.asarray(inputs["bo"], f32)
    w1 = np.asarray(inputs["mlp_w1"], f32); b1 = np.asarray(inputs["mlp_b1"], f32)
    w2 = np.asarray(inputs["mlp_w2"], f32); b2 = np.asarray(inputs["mlp_b2"], f32)
    rin_w = np.asarray(inputs["rin_w"], f32)
    probe = np.asarray(inputs["probe"], f32).reshape(D)

    def r128(v):  # (n*128,) -> (128, n) partition-major
        return np.ascontiguousarray(v.reshape(-1, P).T)

    shared = {
        "bo_r": bo.reshape(1, D),
        "ln_g_r": r128(np.asarray(inputs["ln_g"], f32)),
        "ln_b_r": r128(np.asarray(inputs["ln_b"], f32)),
        "b2_r": b2.reshape(1, D),
        "probe_r": r128(probe).astype(np.float16),
        "four_w2": np.concatenate(
            [np.asarray(inputs["four_w"], f32).reshape(TD // 2, 1)] * 2),
        "phase2": np.concatenate(
            [np.full((TD // 2, 1), np.pi / 2, f32),
             np.zeros((TD // 2, 1), f32)]),
        "timeT": np.ascontiguousarray(np.asarray(inputs["time"], f32).T),
        "naT": np.ascontiguousarray(np.asarray(inputs["noisy_actions"], f32).T).astype(np.float16),
        "cond_w1": np.asarray(inputs["cond_w1"], f32).astype(np.float16),
        "cond_b1c": np.asarray(inputs["cond_b1"], f32).reshape(-1, 1),
        "cond_w2": np.asarray(inputs["cond_w2"], f32).astype(np.float16),
        "cond_b2c": np.asarray(inputs["cond_b2"], f32).reshape(-1, 1),
        "rin_cond": np.ascontiguousarray(rin_w[0:TD]).astype(np.float16),
        "rin_pool": np.ascontiguousarray(rin_w[TD:TD + D]).astype(np.float16),
        "rin_na": np.ascontiguousarray(rin_w[TD + D:]).astype(np.float16),
        "rin_b": np.asarray(inputs["rin_b"], f32).reshape(1, HID),
        "blk_ln_g": np.asarray(inputs["blk_ln_g"], f32),
        "blk_ln_b": np.asarray(inputs["blk_ln_b"], f32),
        "blk_w1": np.asarray(inputs["blk_w1"], f32).astype(np.float16),
        "blk_b1": np.asarray(inputs["blk_b1"], f32),
        "blk_w2": np.asarray(inputs["blk_w2"], f32).astype(np.float16),
        "blk_b2": np.asarray(inputs["blk_b2"], f32),
        "out_w": np.asarray(inputs["out_w"], f32).astype(np.float16),
        "out_bc": np.asarray(inputs["out_b"], f32).reshape(1, AD),
    }

    in_maps = []
    for i in range(NC):
        hb = slice(i * DH, (i + 1) * DH)
        fb = slice(i * F1S, (i + 1) * F1S)
        m = dict(shared)
        m["llm"] = llm_full[i].astype(np.float16)
        m["llmT"] = np.ascontiguousarray(llm_full[i].T).astype(np.float16)
        m["wq_s"] = np.ascontiguousarray(wq[:, hb]).astype(np.float16)
        m["bq_s"] = np.ascontiguousarray(bq[hb]).reshape(1, DH)
        m["wkT_s"] = np.ascontiguousarray(wk[:, hb].T).astype(np.float16)
        m["wv_s"] = np.ascontiguousarray(wv[:, hb]).astype(np.float16)
        m["bv_s"] = np.ascontiguousarray(bv[hb]).reshape(1, DH)
        m["wo_s"] = np.ascontiguousarray(wo[hb, :]).astype(np.float16)
        m["w1_s"] = np.ascontiguousarray(w1[:, fb]).astype(np.float16)
        m["b1_s"] = np.ascontiguousarray(b1[fb]).reshape(1, F1S)
        m["w2_s"] = np.ascontiguousarray(w2[fb, :]).astype(np.float16)
        in_maps.append(m)
    return in_maps


def kernel(**inputs):
    nc = _get_nc()
    in_maps = _prep_in_maps(inputs)
    r = run_bass_kernel_spmd(nc, in_maps, core_ids=list(range(NC)))
    return np.ascontiguousarray(r.results[0]["res"]).astype(np.float32)


def run_traced(**inputs):
    """Like kernel() but with NTFF tracing; returns (output, results)."""
    nc = _get_nc()
    in_maps = _prep_in_maps(inputs)
    r = run_bass_kernel_spmd(nc, in_maps, core_ids=list(range(NC)), trace=True)
    return np.ascontiguousarray(r.results[0]["res"]).astype(np.float32), r


# revision 26
# speedup vs baseline: 1.6400x; 1.0711x over previous
"""Trainium2 Bass kernel for nn_DiffusionActionHead (B=8, S=2048, D=4096).

Strategy (8 NeuronCores):
  - Data-parallel over batch for everything touching llm_output (32 MiB/core).
  - Tensor-parallel weight reads: core i reads column-slice i of wq/wk/wv,
    row-slice i of wo, column/row slice i of mlp_w1/mlp_w2 (~96 MiB of
    weights split 8 ways), tiny diffusion tail replicated.
  - MAP-head attention with q_len=1 is collapsed algebraically:
        scores[s,h] = llm[s,:] . U[:,h],   U[:,h] = wk[:,hb] @ q_h / sqrt(DH)
        pooled[h,:] = softmax(scores)[h,:] @ llm
        ctx[hb]     = pooled[h,:] @ wv[:,hb] + bv[hb]
    (bk shifts scores by a per-head constant -> cancels in softmax.)
  - 4 small collectives: AllGather(U cols), AllToAll(pooled, head<->batch),
    AllReduce(attn_out partial), AllReduce(mlp partial).
  - Large matmuls run in fp16 (1 cyc/row on PE, half the HBM bytes); all
    accumulation, softmax, layernorms and residuals stay fp32.
  - Biases are folded into the PSUM accumulations via a ones-row matmul;
    additive biases of AllReduce'd partials are pre-divided by 8 on host.
  - Activations are kept feature-on-partition ("transposed") so every big
    matmul streams its weight slice in natural DRAM layout as the moving
    operand; llm itself is passed in both layouts (llmT host-transposed).
  - Two HWDGE queues: sync carries the llm streams, scalar carries the
    weight streams, so a stalled stream never head-of-line-blocks the other.
"""

import numpy as np
import sys

if "/opt/trn_rl_repo" not in sys.path:
    sys.path.insert(0, "/opt/trn_rl_repo")

import concourse.bass as bass
import concourse.tile as tile
from concourse import bacc, mybir
from concourse.masks import make_identity
from concourse.bass_utils import run_bass_kernel_spmd

F32 = mybir.dt.float32
F16 = mybir.dt.float16
AF = mybir.ActivationFunctionType
ALU = mybir.AluOpType

B, S, D = 8, 2048, 4096
H, AD, TD, HID, NBLK = 8, 7, 32, 256, 3
DH = D // H            # 512
NC = 8                 # cores
P = 128
SC = S // P            # 16 S-chunks
DC = D // P            # 32 D-chunks
HD2 = D // 2           # 2048 (half width -> 4-bank PSUM tiles)
F1S = 4 * D // NC      # 2048 per-core hidden cols of mlp_w1
HC = HID // P          # 2
RSQRT_DH = 1.0 / float(np.sqrt(DH))
TWO_PI = 2.0 * float(np.pi)


def _bcast(src_ap, nparts):
    """Partition-broadcast a (1, N) DRAM AP to (nparts, N)."""
    ap = src_ap
    assert ap.shape[0] == 1, ap.shape
    return bass.AP(tensor=ap.tensor, offset=ap.offset,
                   ap=[[0, nparts]] + [list(x) for x in ap.ap[1:]])


def build_program():
    nc = bacc.Bacc("TRN2", target_bir_lowering=False, debug=False,
                   num_devices=NC)
    t = {}

    def din(name, shape, dtype=F32):
        t[name] = nc.dram_tensor(name, shape, dtype, kind="ExternalInput")

    din("llm", [S, D], F16); din("llmT", [D, S], F16)
    din("wq_s", [D, DH], F16); din("bq_s", [1, DH])
    din("wkT_s", [DH, D], F16)
    din("wv_s", [D, DH], F16); din("bv16", [1, DH], F16)
    din("wo_s", [DH, D], F16); din("bo16", [1, D], F16)        # bo/8
    din("ln_g_r", [P, DC]); din("ln_b_r", [P, DC])
    din("w1_s", [D, F1S], F16); din("b116", [1, F1S], F16)
    din("w2_s", [F1S, D], F16); din("b216", [1, D], F16)       # b2/8
    din("probe_r", [P, DC], F16)
    din("four_w2", [TD, 1]); din("phase2", [TD, 1])
    din("timeT", [1, B]); din("naT", [AD, B], F16)
    din("cond_w1", [TD, 2 * TD], F16); din("cond_b1c", [2 * TD, 1])
    din("cond_w2", [2 * TD, TD], F16); din("cond_b2c", [TD, 1])
    din("rin_cond", [TD, HID], F16); din("rin_pool", [D, HID], F16)
    din("rin_na", [AD, HID], F16); din("rb16", [1, HID], F16)
    din("blk_g_r", [NBLK, P, HC]); din("blk_b_r", [NBLK, P, HC])
    din("blk_w1", [NBLK, HID, 4 * HID], F16)
    din("blk_b1_16", [NBLK, 4 * HID], F16)
    din("blk_w2", [NBLK, 4 * HID, HID], F16)
    din("blk_b2_16", [NBLK, HID], F16)
    din("out_w", [HID, AD], F16); din("out_bc", [1, AD])
    t["res"] = nc.dram_tensor("res", [B, AD], F32, kind="ExternalOutput")

    # collective bounce buffers (internal DRAM; AG/AR outputs in Shared space)
    t["cc_u_in"] = nc.dram_tensor("cc_u_in", [1, D], F32)
    t["cc_u_out"] = nc.dram_tensor("cc_u_out", [NC, D], F32, addr_space="Shared")
    t["cc_pool_in"] = nc.dram_tensor("cc_pool_in", [H, D], F32)
    t["cc_pool_out"] = nc.dram_tensor("cc_pool_out", [B, D], F32)
    t["cc_attn_in"] = nc.dram_tensor("cc_attn_in", [B, D], F32)
    t["cc_attn_out"] = nc.dram_tensor("cc_attn_out", [B, D], F32,
                                      addr_space="Shared")
    t["cc_mlp_in"] = nc.dram_tensor("cc_mlp_in", [B, D], F32)
    t["cc_mlp_out"] = nc.dram_tensor("cc_mlp_out", [B, D], F32,
                                     addr_space="Shared")

    with tile.TileContext(nc) as tc:
        import contextlib
        with contextlib.ExitStack() as ctx:
            _build(nc, tc, t, ctx)
    nc.finalize()
    return nc


def _build(nc, tc, t, ctx):
    GROUPS = [list(range(NC))]

    singles = ctx.enter_context(tc.tile_pool(name="singles", bufs=1))
    llm_pool = ctx.enter_context(tc.tile_pool(name="llm_pool", bufs=6))
    llmT_pool = ctx.enter_context(tc.tile_pool(name="llmT_pool", bufs=8))
    wst = ctx.enter_context(tc.tile_pool(name="wst", bufs=8))
    nat16 = ctx.enter_context(tc.tile_pool(name="nat16", bufs=2))
    nat8 = ctx.enter_context(tc.tile_pool(name="nat8", bufs=2))
    psA = ctx.enter_context(tc.tile_pool(name="psA", bufs=1, space="PSUM"))
    psB = ctx.enter_context(tc.tile_pool(name="psB", bufs=2, space="PSUM"))
    psC = ctx.enter_context(tc.tile_pool(name="psC", bufs=2, space="PSUM"))

    ident = singles.tile([P, P], F32)
    make_identity(nc, ident)
    eps_sb = singles.tile([P, 1], F32)
    nc.vector.memset(eps_sb[:], 1e-5)
    ones8 = singles.tile([1, 8], F16)
    nc.vector.memset(ones8[:], 1.0)

    def evict(dst, src):
        nc.vector.tensor_copy(out=dst, in_=src)

    def t_nat_to_T(src_nat, dst_T, nchunks, npart, uid):
        """(npart, nchunks*128) sbuf -> (128, nchunks, npart) sbuf via PE."""
        for c in range(nchunks):
            ps = psB.tile([P, 8], F32, tag="tp8", name=f"tp_{uid}_{c}")
            nc.tensor.transpose(ps[:, :npart], src_nat[:, c * P:(c + 1) * P],
                                ident[:npart, :npart])
            evict(dst_T[:, c, :], ps[:, :npart])

    def bias_mm(ps, bias_row, n_total, stop=True):
        """Add a (1, n_total) f16 bias row into psum (8, n_total) via ones-row
        matmuls, 512 cols per matmul (moving-dim limit)."""
        nch = (n_total + 511) // 512
        for n in range(nch):
            w = min(512, n_total - n * 512)
            nc.tensor.matmul(ps[:, n * 512:n * 512 + w], ones8[:, :B],
                             bias_row[:, n * 512:n * 512 + w],
                             start=False, stop=(stop and n == nch - 1))

    def layernorm_nat(x_nat, npart, n, y_nat, uid):
        """y = (x - mean) / sqrt(var + eps) over the free dim of (npart, n)."""
        nsub = max(1, n // 512)
        st = nat8.tile([npart, nsub, nc.vector.BN_STATS_DIM], F32, tag="lnst",
                       name=f"lnst_{uid}")
        xg = x_nat.rearrange("p (a b) -> p a b", a=nsub)
        for g in range(nsub):
            nc.vector.bn_stats(out=st[:, g, :], in_=xg[:, g, :])
        mv = nat8.tile([npart, nc.vector.BN_AGGR_DIM], F32, tag="lnmv",
                       name=f"lnmv_{uid}")
        nc.vector.bn_aggr(out=mv[:], in_=st[:])
        std = nat8.tile([npart, 1], F32, tag="lnsd", name=f"lnsd_{uid}")
        nc.scalar.activation(out=std[:], in_=mv[:, 1:2], func=AF.Sqrt,
                             bias=eps_sb[:npart, :])
        nc.vector.reciprocal(out=std[:], in_=std[:])
        nc.vector.tensor_scalar(out=y_nat, in0=x_nat, scalar1=mv[:, 0:1],
                                scalar2=std[:], op0=ALU.subtract, op1=ALU.mult)

    # =======================================================================
    # STEP 0: small constants, bias rows, tail weights — all prefetched
    # early on idle queues so the tail phase never waits on them.
    # =======================================================================
    probe_sb = singles.tile([P, DC], F16)
    nc.sync.dma_start(out=probe_sb[:], in_=t["probe_r"][:])
    bq_sb = singles.tile([1, DH], F32)
    nc.sync.dma_start(out=bq_sb[:], in_=t["bq_s"][:])
    bv_sb = singles.tile([1, DH], F16)
    nc.gpsimd.dma_start(out=bv_sb[:], in_=t["bv16"][:])
    bo_sb = singles.tile([1, D], F16)
    nc.gpsimd.dma_start(out=bo_sb[:], in_=t["bo16"][:])
    b1_sb = singles.tile([1, F1S], F16)
    nc.gpsimd.dma_start(out=b1_sb[:], in_=t["b116"][:])
    b2_sb = singles.tile([1, D], F16)
    nc.gpsimd.dma_start(out=b2_sb[:], in_=t["b216"][:])
    rb_sb = singles.tile([1, HID], F16)
    nc.gpsimd.dma_start(out=rb_sb[:], in_=t["rb16"][:])
    bb1_sb = singles.tile([1, NBLK, 4 * HID], F16)
    nc.gpsimd.dma_start(out=bb1_sb[:], in_=t["blk_b1_16"][:].rearrange("n f -> (n f)")[None, :])
    bb2_sb = singles.tile([1, NBLK, HID], F16)
    nc.gpsimd.dma_start(out=bb2_sb[:], in_=t["blk_b2_16"][:].rearrange("n f -> (n f)")[None, :])
    lng_sb = singles.tile([P, DC], F32)
    nc.sync.dma_start(out=lng_sb[:], in_=t["ln_g_r"][:])
    lnb_sb = singles.tile([P, DC], F32)
    nc.sync.dma_start(out=lnb_sb[:], in_=t["ln_b_r"][:])
    bgr_sb = singles.tile([P, NBLK, HC], F32)
    nc.sync.dma_start(out=bgr_sb[:],
                      in_=t["blk_g_r"][:].rearrange("n p c -> p n c"))
    bbr_sb = singles.tile([P, NBLK, HC], F32)
    nc.sync.dma_start(out=bbr_sb[:],
                      in_=t["blk_b_r"][:].rearrange("n p c -> p n c"))
    rc_sb = singles.tile([TD, HID], F16)
    nc.scalar.dma_start(out=rc_sb[:], in_=t["rin_cond"][:])
    rna_sb = singles.tile([AD, HID], F16)
    nc.scalar.dma_start(out=rna_sb[:], in_=t["rin_na"][:])
    naT_sb = singles.tile([AD, B], F16)
    nc.sync.dma_start(out=naT_sb[:], in_=t["naT"][:])
    ow_sb = singles.tile([P, HC, AD], F16)
    nc.sync.dma_start(out=ow_sb[:],
                      in_=t["out_w"][:].rearrange("(c p) a -> p c a", p=P))
    ob_bc = singles.tile([B, AD], F32)
    nc.gpsimd.dma_start(out=ob_bc[:], in_=_bcast(t["out_bc"][:], B))

    # =======================================================================
    # STEP 1: q = (probe @ wq_s + bq) / sqrt(DH)    -> (1, 512) natural
    # wq is streamed in 8 half-MiB DMAs (4 k-chunks each) on the scalar ring.
    # =======================================================================
    q_nat = singles.tile([1, DH], F32)
    ps_q = psC.tile([1, DH], F32, tag="vec", name="ps_q")
    wq_r = t["wq_s"].rearrange("(c p) n -> p c n", p=P)
    for g in range(8):
        wt = wst.tile([P, 4, DH], F16, tag="wst", name=f"wq_g{g}")
        nc.scalar.dma_start(out=wt[:], in_=wq_r[:, 4 * g:4 * g + 4, :])
        for j in range(4):
            k = 4 * g + j
            nc.tensor.matmul(ps_q[:], probe_sb[:, k:k + 1], wt[:, j, :],
                             start=(k == 0), stop=(k == DC - 1))
    nc.vector.tensor_add(out=q_nat[:], in0=ps_q[:], in1=bq_sb[:])
    nc.vector.tensor_scalar_mul(out=q_nat[:], in0=q_nat[:], scalar1=RSQRT_DH)

    qT = singles.tile([P, DH // P], F16)  # (128, 4)
    for c in range(DH // P):
        ps = psB.tile([P, 8], F32, tag="tp8", name=f"tp_q_{c}")
        nc.tensor.transpose(ps[:, :1], q_nat[:, c * P:(c + 1) * P], ident[:1, :1])
        evict(qT[:, c:c + 1], ps[:, :1])

    # =======================================================================
    # STEP 2: U column of this core's head: U = wkT_s.T @ q~  -> (1, 4096)
    #         AllGather -> cc_u_out (8, 4096) = U.T with one row per head
    # =======================================================================
    u_nat = nat16.tile([1, D], F32, tag="nat16", name="u_nat")
    for nhalf in range(2):
        wk_tiles = []
        for k in range(DH // P):
            wt = wst.tile([P, HD2], F16, tag="wst", name=f"wk_t{nhalf}_{k}")
            nc.scalar.dma_start(
                out=wt[:],
                in_=t["wkT_s"][k * P:(k + 1) * P, nhalf * HD2:(nhalf + 1) * HD2])
            wk_tiles.append(wt)
        for ncol in range(4):
            n0 = nhalf * 4 + ncol
            ps_u = psC.tile([1, DH], F32, tag="vec", name=f"ps_u_{n0}")
            for k in range(DH // P):
                nc.tensor.matmul(
                    ps_u[:], qT[:, k:k + 1],
                    wk_tiles[k][:, ncol * DH:(ncol + 1) * DH],
                    start=(k == 0), stop=(k == DH // P - 1))
            evict(u_nat[:, n0 * DH:(n0 + 1) * DH], ps_u[:])

    nc.gpsimd.dma_start(out=t["cc_u_in"][:], in_=u_nat[:])
    nc.gpsimd.collective_compute(
        "AllGather", ALU.bypass, replica_groups=GROUPS,
        ins=[t["cc_u_in"][:].opt()], outs=[t["cc_u_out"][:].opt()])

    # ---- cond path (fourier + tiny mlp) — independent of everything above,
    # computed here so it is off the critical path of the tail.
    fw_sb = singles.tile([TD, 1], F32)
    nc.sync.dma_start(out=fw_sb[:], in_=t["four_w2"][:])
    ph_sb = singles.tile([TD, 1], F32)
    nc.sync.dma_start(out=ph_sb[:], in_=t["phase2"][:])
    tb32 = singles.tile([TD, B], F32)
    nc.gpsimd.dma_start(out=tb32[:], in_=_bcast(t["timeT"][:], TD))
    fu = singles.tile([TD, B], F32)
    nc.vector.tensor_scalar_mul(out=fu[:], in0=tb32[:], scalar1=fw_sb[:])
    # exact range reduction: sin/cos have period 1 in fu, so subtract the
    # integer part via an f32->i32->f32 round-trip (|fu| < ~64 here).
    fi = singles.tile([TD, B], mybir.dt.int32)
    nc.vector.tensor_copy(out=fi[:], in_=fu[:])
    fif = singles.tile([TD, B], F32)
    nc.vector.tensor_copy(out=fif[:], in_=fi[:])
    nc.vector.tensor_sub(out=fu[:], in0=fu[:], in1=fif[:])
    ffT = singles.tile([TD, B], F16)
    nc.scalar.activation(out=ffT[:], in_=fu[:], func=AF.Sin,
                         scale=TWO_PI, bias=ph_sb[:])
    cw1_sb = singles.tile([TD, 2 * TD], F16)
    nc.scalar.dma_start(out=cw1_sb[:], in_=t["cond_w1"][:])
    cb1_sb = singles.tile([2 * TD, 1], F32)
    nc.sync.dma_start(out=cb1_sb[:], in_=t["cond_b1c"][:])
    cw2_sb = singles.tile([2 * TD, TD], F16)
    nc.scalar.dma_start(out=cw2_sb[:], in_=t["cond_w2"][:])
    cb2_sb = singles.tile([TD, 1], F32)
    nc.sync.dma_start(out=cb2_sb[:], in_=t["cond_b2c"][:])
    ps_c1 = psB.tile([P, 8], F32, tag="tp8", name="ps_c1")
    nc.tensor.matmul(ps_c1[:2 * TD, :B], cw1_sb[:], ffT[:], start=True, stop=True)
    c1 = singles.tile([2 * TD, B], F16)
    nc.scalar.activation(out=c1[:], in_=ps_c1[:2 * TD, :B], func=AF.Silu,
                         bias=cb1_sb[:])
    ps_c2 = psB.tile([P, 8], F32, tag="tp8", name="ps_c2")
    nc.tensor.matmul(ps_c2[:TD, :B], cw2_sb[:], c1[:], start=True, stop=True)
    condT = singles.tile([TD, B], F16)
    nc.scalar.activation(out=condT[:], in_=ps_c2[:TD, :B], func=AF.Identity,
                         bias=cb2_sb[:])

    # ---- read back U.T (8, 4096), transpose to (128, 32, 8), cast to f16
    uh_nat = nat16.tile([H, D], F32, tag="nat16", name="uh_nat")
    nc.sync.dma_start(out=uh_nat[:], in_=t["cc_u_out"][:])
    u_f16 = singles.tile([P, DC, H], F16)
    for c in range(DC):
        ps = psB.tile([P, 8], F32, tag="tp8", name=f"tp_u_{c}")
        nc.tensor.transpose(ps[:, :H], uh_nat[:, c * P:(c + 1) * P],
                            ident[:H, :H])
        evict(u_f16[:, c, :], ps[:, :H])

    # =======================================================================
    # STEP 3: scoresT (8, 2048) = U.T @ llmT  (fp16 inputs, fp32 accum)
    # =======================================================================
    ps_sc = psA.tile([H, S], F32, tag="big", name="ps_sc")
    for k in range(DC):
        lt = llmT_pool.tile([P, S], F16, tag="llmT", name=f"llmT_t{k}")
        nc.sync.dma_start(out=lt[:], in_=t["llmT"][k * P:(k + 1) * P, :])
        for n in range(S // 512):
            nc.tensor.matmul(ps_sc[:, n * 512:(n + 1) * 512],
                             u_f16[:, k, :], lt[:, n * 512:(n + 1) * 512],
                             start=(k == 0), stop=(k == DC - 1))

    # =======================================================================
    # STEP 4: softmax over S. Max-subtraction is skipped deliberately:
    # softmax is shift-invariant and |scores| here is < ~1, so exp() is
    # perfectly conditioned; the result is mathematically identical.
    # =======================================================================
    p_nat = nat8.tile([H, S], F32, tag="nat8", name="p_nat")
    nc.scalar.activation(out=p_nat[:], in_=ps_sc[:], func=AF.Exp)
    den = singles.tile([H, 1], F32)
    nc.vector.reduce_sum(out=den[:], in_=p_nat[:], axis=mybir.AxisListType.X)
    nc.vector.reciprocal(out=den[:], in_=den[:])
    nc.vector.tensor_scalar_mul(out=p_nat[:], in0=p_nat[:], scalar1=den[:])
    pT = singles.tile([P, SC, H], F16)
    t_nat_to_T(p_nat, pT, SC, H, "p")

    # =======================================================================
    # STEP 5: pooled (8, 4096) = pT.T @ llm ; AllToAll (head <-> batch)
    # =======================================================================
    pooled_nat = nat16.tile([H, D], F32, tag="nat16", name="pooled_nat")
    for half in range(2):
        ps_p = psA.tile([H, HD2], F32, tag="big", name=f"ps_pool_{half}")
        for s in range(SC):
            lt = llm_pool.tile([P, HD2], F16, tag="llm", name=f"llm_t{half}_{s}")
            nc.sync.dma_start(
                out=lt[:],
                in_=t["llm"][s * P:(s + 1) * P, half * HD2:(half + 1) * HD2])
            for n in range(HD2 // 512):
                nc.tensor.matmul(ps_p[:, n * 512:(n + 1) * 512],
                                 pT[:, s, :], lt[:, n * 512:(n + 1) * 512],
                                 start=(s == 0), stop=(s == SC - 1))
        evict(pooled_nat[:, half * HD2:(half + 1) * HD2], ps_p[:])

    nc.gpsimd.dma_start(out=t["cc_pool_in"][:], in_=pooled_nat[:])
    nc.gpsimd.collective_compute(
        "AllToAll", ALU.bypass, replica_groups=GROUPS,
        ins=[t["cc_pool_in"][:].opt()], outs=[t["cc_pool_out"][:].opt()])

    # =======================================================================
    # STEP 6: ctx for this core's head, all batches: (8, 512) = poolh@wv + bv
    # =======================================================================
    poolh_nat = nat16.tile([B, D], F32, tag="nat16", name="poolh_nat")
    nc.sync.dma_start(out=poolh_nat[:], in_=t["cc_pool_out"][:])
    poolhT = singles.tile([P, DC, B], F16)
    t_nat_to_T(poolh_nat, poolhT, DC, B, "ph")

    ps_cx = psA.tile([B, DH], F32, tag="big", name="ps_cx")
    wv_r = t["wv_s"].rearrange("(c p) n -> p c n", p=P)
    for g in range(8):
        wt = wst.tile([P, 4, DH], F16, tag="wst", name=f"wv_g{g}")
        nc.scalar.dma_start(out=wt[:], in_=wv_r[:, 4 * g:4 * g + 4, :])
        for j in range(4):
            k = 4 * g + j
            nc.tensor.matmul(ps_cx[:], poolhT[:, k, :], wt[:, j, :],
                             start=(k == 0), stop=False)
    bias_mm(ps_cx, bv_sb, DH)
    ctx_nat = nat8.tile([B, DH], F32, tag="nat8", name="ctx_nat")
    evict(ctx_nat[:], ps_cx[:])
    ctxT = singles.tile([P, DH // P, B], F16)
    t_nat_to_T(ctx_nat, ctxT, DH // P, B, "cx")

    # =======================================================================
    # STEP 7: attn partial (8, 4096) = ctx @ wo_s + bo/8 ; AllReduce
    # =======================================================================
    attn_part = nat16.tile([B, D], F32, tag="nat16", name="attn_part")
    for half in range(2):
        ps_a = psA.tile([B, HD2], F32, tag="big", name=f"ps_attn_{half}")
        for k in range(DH // P):
            wt = wst.tile([P, HD2], F16, tag="wst", name=f"wo_t{half}_{k}")
            nc.scalar.dma_start(
                out=wt[:],
                in_=t["wo_s"][k * P:(k + 1) * P, half * HD2:(half + 1) * HD2])
            for n in range(HD2 // 512):
                nc.tensor.matmul(ps_a[:, n * 512:(n + 1) * 512],
                                 ctxT[:, k, :], wt[:, n * 512:(n + 1) * 512],
                                 start=(k == 0), stop=False)
        bias_mm(ps_a, bo_sb[:, half * HD2:(half + 1) * HD2], HD2)
        evict(attn_part[:, half * HD2:(half + 1) * HD2], ps_a[:])
    nc.gpsimd.dma_start(out=t["cc_attn_in"][:], in_=attn_part[:])
    nc.gpsimd.collective_compute(
        "AllReduce", ALU.add, replica_groups=GROUPS,
        ins=[t["cc_attn_in"][:].opt()], outs=[t["cc_attn_out"][:].opt()])

    # =======================================================================
    # STEP 8: y = LN(attn_out)*g+b ; mlp partial (+b1, gelu, @w2 + b2/8) ; AR
    # =======================================================================
    attn_nat = singles.tile([B, D], F32)  # persists (residual)
    nc.sync.dma_start(out=attn_nat[:], in_=t["cc_attn_out"][:])

    y_nat = nat16.tile([B, D], F32, tag="nat16", name="y_nat")
    layernorm_nat(attn_nat[:], B, D, y_nat[:], "ln0")
    yT = singles.tile([P, DC, B], F16)
    t_nat_to_T(y_nat, yT, DC, B, "y")
    # LN affine in T layout (gamma/beta become per-partition scalars)
    for c in range(DC):
        nc.vector.tensor_scalar(out=yT[:, c, :], in0=yT[:, c, :],
                                scalar1=lng_sb[:, c:c + 1],
                                scalar2=lnb_sb[:, c:c + 1],
                                op0=ALU.mult, op1=ALU.add)

    # mm1: h1 (8, 2048) = y @ w1_s + b1 ; exact gelu straight off PSUM
    ps_h1 = psA.tile([B, F1S], F32, tag="big", name="ps_h1")
    for k in range(DC):
        wt = wst.tile([P, F1S], F16, tag="wst", name=f"w1_t{k}")
        nc.scalar.dma_start(out=wt[:], in_=t["w1_s"][k * P:(k + 1) * P, :])
        for n in range(F1S // 512):
            nc.tensor.matmul(ps_h1[:, n * 512:(n + 1) * 512],
                             yT[:, k, :], wt[:, n * 512:(n + 1) * 512],
                             start=(k == 0), stop=False)
    bias_mm(ps_h1, b1_sb, F1S)
    g_nat = nat8.tile([B, F1S], F32, tag="nat8", name="g_nat")
    nc.scalar.activation(out=g_nat[:], in_=ps_h1[:], func=AF.Gelu)
    gT = singles.tile([P, F1S // P, B], F16)
    t_nat_to_T(g_nat, gT, F1S // P, B, "g")

    # mm2: h2 partial (8, 4096) = g @ w2_s + b2/8 ; AllReduce
    h2_nat = nat16.tile([B, D], F32, tag="nat16", name="h2_nat")
    for half in range(2):
        ps_h2 = psA.tile([B, HD2], F32, tag="big", name=f"ps_h2_{half}")
        for k in range(F1S // P):
            wt = wst.tile([P, HD2], F16, tag="wst", name=f"w2_t{half}_{k}")
            nc.scalar.dma_start(
                out=wt[:],
                in_=t["w2_s"][k * P:(k + 1) * P, half * HD2:(half + 1) * HD2])
            for n in range(HD2 // 512):
                nc.tensor.matmul(ps_h2[:, n * 512:(n + 1) * 512],
                                 gT[:, k, :], wt[:, n * 512:(n + 1) * 512],
                                 start=(k == 0), stop=False)
        bias_mm(ps_h2, b2_sb[:, half * HD2:(half + 1) * HD2], HD2)
        evict(h2_nat[:, half * HD2:(half + 1) * HD2], ps_h2[:])
    nc.gpsimd.dma_start(out=t["cc_mlp_in"][:], in_=h2_nat[:])
    nc.gpsimd.collective_compute(
        "AllReduce", ALU.add, replica_groups=GROUPS,
        ins=[t["cc_mlp_in"][:].opt()], outs=[t["cc_mlp_out"][:].opt()])

    # =======================================================================
    # STEP 9: x_pool = attn_out + h ; diffusion tail (replicated on all cores)
    # =======================================================================
    hug = nat16.tile([B, D], F32, tag="nat16", name="hug")
    nc.sync.dma_start(out=hug[:], in_=t["cc_mlp_out"][:])
    nc.vector.tensor_add(out=attn_nat[:], in0=attn_nat[:], in1=hug[:])
    xpT = singles.tile([P, DC, B], F16)
    t_nat_to_T(attn_nat, xpT, DC, B, "xp")

    # x0 (8, 256) = x_pool@rin_pool + cond@rin_cond + na@rin_na + rin_b
    ps_x0 = psA.tile([B, HID], F32, tag="big", name="ps_x0")
    for k in range(DC):
        wt = wst.tile([P, HID], F16, tag="wst", name=f"rp_t{k}")
        nc.scalar.dma_start(out=wt[:], in_=t["rin_pool"][k * P:(k + 1) * P, :])
        nc.tensor.matmul(ps_x0[:], xpT[:, k, :], wt[:], start=(k == 0),
                         stop=False)
    nc.tensor.matmul(ps_x0[:], condT[:], rc_sb[:], start=False, stop=False)
    nc.tensor.matmul(ps_x0[:], naT_sb[:], rna_sb[:], start=False, stop=False)
    bias_mm(ps_x0, rb_sb, HID)
    x_nat = singles.tile([B, HID], F32)
    evict(x_nat[:], ps_x0[:])

    # ---- 3 residual blocks ----
    for i in range(NBLK):
        xn = singles.tile([B, HID], F32, name=f"xn_{i}")
        layernorm_nat(x_nat[:], B, HID, xn[:], f"lnb{i}")
        xnT = singles.tile([P, HC, B], F16, name=f"xnT_{i}")
        t_nat_to_T(xn, xnT, HC, B, f"xn{i}")
        for c in range(HC):  # LN affine in T layout
            nc.vector.tensor_scalar(out=xnT[:, c, :], in0=xnT[:, c, :],
                                    scalar1=bgr_sb[:, i, c:c + 1],
                                    scalar2=bbr_sb[:, i, c:c + 1],
                                    op0=ALU.mult, op1=ALU.add)

        ps_bh = psA.tile([B, 4 * HID], F32, tag="big", name=f"ps_bh_{i}")
        for k in range(HC):
            wt = wst.tile([P, 4 * HID], F16, tag="wst", name=f"bw1_t{i}_{k}")
            nc.scalar.dma_start(out=wt[:], in_=t["blk_w1"][i, k * P:(k + 1) * P, :])
            for n in range(4 * HID // 512):
                nc.tensor.matmul(ps_bh[:, n * 512:(n + 1) * 512],
                                 xnT[:, k, :], wt[:, n * 512:(n + 1) * 512],
                                 start=(k == 0), stop=False)
        bias_mm(ps_bh, bb1_sb[:, i, :], 4 * HID)
        hb = nat8.tile([B, 4 * HID], F32, tag="nat8", name=f"hb_{i}")
        nc.scalar.activation(out=hb[:], in_=ps_bh[:], func=AF.Silu)
        hbT = singles.tile([P, 4 * HID // P, B], F16, name=f"hbT_{i}")
        t_nat_to_T(hb, hbT, 4 * HID // P, B, f"hb{i}")

        ps_bo = psA.tile([B, HID], F32, tag="big", name=f"ps_bo_{i}")
        for k in range(4 * HID // P):
            wt = wst.tile([P, HID], F16, tag="wst", name=f"bw2_t{i}_{k}")
            nc.scalar.dma_start(out=wt[:], in_=t["blk_w2"][i, k * P:(k + 1) * P, :])
            nc.tensor.matmul(ps_bo[:], hbT[:, k, :], wt[:],
                             start=(k == 0), stop=False)
        bias_mm(ps_bo, bb2_sb[:, i, :], HID)
        nc.vector.tensor_add(out=x_nat[:], in0=x_nat[:], in1=ps_bo[:])

    # ---- final: res (8, 7) = swish(x) @ out_w + out_b
    nc.scalar.activation(out=x_nat[:], in_=x_nat[:], func=AF.Silu)
    xsT = singles.tile([P, HC, B], F16)
    t_nat_to_T(x_nat, xsT, HC, B, "xs")
    ps_o = psB.tile([P, 8], F32, tag="tp8", name="ps_o")
    for k in range(HC):
        nc.tensor.matmul(ps_o[:B, :AD], xsT[:, k, :], ow_sb[:, k, :],
                         start=(k == 0), stop=(k == HC - 1))
    out_sb = singles.tile([B, AD], F32)
    nc.vector.tensor_add(out=out_sb[:], in0=ps_o[:B, :AD], in1=ob_bc[:])
    nc.sync.dma_start(out=t["res"][:], in_=out_sb[:])


_CACHED_NC = None


def _get_nc():
    global _CACHED_NC
    if _CACHED_NC is None:
        _CACHED_NC = build_program()
    return _CACHED_NC


def _prep_in_maps(inputs):
    f32 = np.float32
    f16 = np.float16
    llm_full = np.ascontiguousarray(np.asarray(inputs["llm_output"], dtype=f32))
    wq = np.asarray(inputs["wq"], f32); wk = np.asarray(inputs["wk"], f32)
    wv = np.asarray(inputs["wv"], f32); wo = np.asarray(inputs["wo"], f32)
    bq = np.asarray(inputs["bq"], f32); bv = np.asarray(inputs["bv"], f32)
    bo = np.asarray(inputs["bo"], f32)
    w1 = np.asarray(inputs["mlp_w1"], f32); b1 = np.asarray(inputs["mlp_b1"], f32)
    w2 = np.asarray(inputs["mlp_w2"], f32); b2 = np.asarray(inputs["mlp_b2"], f32)
    rin_w = np.asarray(inputs["rin_w"], f32)
    probe = np.asarray(inputs["probe"], f32).reshape(D)

    def r128(v):  # (n*128,) -> (128, n) partition-major
        return np.ascontiguousarray(v.reshape(-1, P).T)

    blk_g = np.asarray(inputs["blk_ln_g"], f32)
    blk_b = np.asarray(inputs["blk_ln_b"], f32)

    shared = {
        "bo16": (bo / NC).astype(f16).reshape(1, D),
        "ln_g_r": r128(np.asarray(inputs["ln_g"], f32)),
        "ln_b_r": r128(np.asarray(inputs["ln_b"], f32)),
        "b216": (b2 / NC).astype(f16).reshape(1, D),
        "probe_r": r128(probe).astype(f16),
        "four_w2": np.concatenate(
            [np.asarray(inputs["four_w"], f32).reshape(TD // 2, 1)] * 2),
        "phase2": np.concatenate(
            [np.full((TD // 2, 1), np.pi / 2, f32),
             np.zeros((TD // 2, 1), f32)]),
        "timeT": np.ascontiguousarray(np.asarray(inputs["time"], f32).T),
        "naT": np.ascontiguousarray(
            np.asarray(inputs["noisy_actions"], f32).T).astype(f16),
        "cond_w1": np.asarray(inputs["cond_w1"], f32).astype(f16),
        "cond_b1c": np.asarray(inputs["cond_b1"], f32).reshape(-1, 1),
        "cond_w2": np.asarray(inputs["cond_w2"], f32).astype(f16),
        "cond_b2c": np.asarray(inputs["cond_b2"], f32).reshape(-1, 1),
        "rin_cond": np.ascontiguousarray(rin_w[0:TD]).astype(f16),
        "rin_pool": np.ascontiguousarray(rin_w[TD:TD + D]).astype(f16),
        "rin_na": np.ascontiguousarray(rin_w[TD + D:]).astype(f16),
        "rb16": np.asarray(inputs["rin_b"], f32).astype(f16).reshape(1, HID),
        "blk_g_r": np.ascontiguousarray(
            blk_g.reshape(NBLK, HC, P).transpose(0, 2, 1)),
        "blk_b_r": np.ascontiguousarray(
            blk_b.reshape(NBLK, HC, P).transpose(0, 2, 1)),
        "blk_w1": np.asarray(inputs["blk_w1"], f32).astype(f16),
        "blk_b1_16": np.asarray(inputs["blk_b1"], f32).astype(f16),
        "blk_w2": np.asarray(inputs["blk_w2"], f32).astype(f16),
        "blk_b2_16": np.asarray(inputs["blk_b2"], f32).astype(f16),
        "out_w": np.asarray(inputs["out_w"], f32).astype(f16),
        "out_bc": np.asarray(inputs["out_b"], f32).reshape(1, AD),
    }

    in_maps = []
    for i in range(NC):
        hb = slice(i * DH, (i + 1) * DH)
        fb = slice(i * F1S, (i + 1) * F1S)
        m = dict(shared)
        m["llm"] = llm_full[i].astype(f16)
        m["llmT"] = np.ascontiguousarray(llm_full[i].T).astype(f16)
        m["wq_s"] = np.ascontiguousarray(wq[:, hb]).astype(f16)
        m["bq_s"] = np.ascontiguousarray(bq[hb]).reshape(1, DH)
        m["wkT_s"] = np.ascontiguousarray(wk[:, hb].T).astype(f16)
        m["wv_s"] = np.ascontiguousarray(wv[:, hb]).astype(f16)
        m["bv16"] = np.ascontiguousarray(bv[hb]).astype(f16).reshape(1, DH)
        m["wo_s"] = np.ascontiguousarray(wo[hb, :]).astype(f16)
        m["w1_s"] = np.ascontiguousarray(w1[:, fb]).astype(f16)
        m["b116"] = np.ascontiguousarray(b1[fb]).astype(f16).reshape(1, F1S)
        m["w2_s"] = np.ascontiguousarray(w2[fb, :]).astype(f16)
        in_maps.append(m)
    return in_maps


def kernel(**inputs):
    nc = _get_nc()
    in_maps = _prep_in_maps(inputs)
    r = run_bass_kernel_spmd(nc, in_maps, core_ids=list(range(NC)))
    return np.ascontiguousarray(r.results[0]["res"]).astype(np.float32)


def run_traced(**inputs):
    """Like kernel() but with NTFF tracing; returns (output, results)."""
    nc = _get_nc()
    in_maps = _prep_in_maps(inputs)
    r = run_bass_kernel_spmd(nc, in_maps, core_ids=list(range(NC)), trace=True)
    return np.ascontiguousarray(r.results[0]["res"]).astype(np.float32), r
